# revision 2
# baseline (speedup 1.0000x reference)
"""Additive-attention kernel for Trainium2 (8 NeuronCores, SPMD).

Problem (per batch b of B=4):
    xt      = x[b].T                                  # (N=512, D=96)
    g1      = xt @ Wg1.T                              # (512, 256)
    g2      = xt @ Wg2.T                              # (512, 256)
    score   = sum_a Wa[a] * tanh(g1[n,a] + g2[m,a] + bg[a])    # (512, 512)
    att     = sigmoid(score + Wa_b + ba)
    out[b]  = att @ xt                                # (512, 96)

Sharding: core c handles batch b = c//2 and query-rows n in
[(c%2)*256, (c%2)*256+256).  Each core computes its full out rows;
host concatenates.

Per-core device algorithm ("column scheme"):
  - g1T[a, n] (own n) and g2T[a, m] (all m) via PE matmuls with K=D=96,
    a-chunks of 128 on partitions; bg folded in via ACT per-partition bias.
  - loop over own n: Z[a, m] = g2T[a, m] + g1T[a, n] via DVE tensor_scalar
    (per-partition scalar add), batches of 4 n; one big ACT Tanh
    [128, 4096] fp32->fp16.
  - scoring: per (n, m-block, a-chunk) matmul with the tanh tile as the
    STATIONARY operand ([K=128a, M=128m]) and Wa chunk [128,1] moving ->
    scoreT[m, n] accumulates as dense PSUM columns (4 banks [128, 256]).
  - sigmoid (+Wa_b+ba bias) PSUM->SBUF fp16 gives attT[m, n] directly.
  - final: out[n, d] = sum_m att[n, m] x[b][d, m]: lhsT = attT chunk,
    rhs = xkT chunk (host-passed x[b].T in fp16), accumulate 4 m-chunks.
"""

import numpy as np

B, D, N, A = 4, 96, 512, 256
NH = N // 2          # query rows per core
G = 4                # n-group per tanh op
NCORES = 8

_cache = {}


def _build_nc():
    import concourse.bacc as bacc
    import concourse.mybir as mybir
    from concourse import tile

    f32 = mybir.dt.float32
    f16 = mybir.dt.float16
    AF = mybir.ActivationFunctionType

    nc = bacc.Bacc("TRN2", target_bir_lowering=False)

    xq = nc.dram_tensor("xq", [D, NH], f32, kind="ExternalInput")
    xk = nc.dram_tensor("xk", [D, N], f32, kind="ExternalInput")
    xkT = nc.dram_tensor("xkT", [N, D], f16, kind="ExternalInput")
    wg1T = nc.dram_tensor("wg1T", [D, A], f32, kind="ExternalInput")
    wg2T = nc.dram_tensor("wg2T", [D, A], f32, kind="ExternalInput")
    waT = nc.dram_tensor("waT", [128, 2], f16, kind="ExternalInput")
    bgc = nc.dram_tensor("bgc", [128, 2], f32, kind="ExternalInput")
    sgb = nc.dram_tensor("sgb", [128, 1], f32, kind="ExternalInput")
    out = nc.dram_tensor("out", [NH, D], f32, kind="ExternalOutput")

    with tile.TileContext(nc) as tc:
        with (
            tc.tile_pool(name="consts", bufs=1) as consts,
            tc.tile_pool(name="gbuf", bufs=1) as gbuf,
            tc.tile_pool(name="zpool", bufs=2) as zpool,
            tc.tile_pool(name="tpool", bufs=2) as tpool,
            tc.tile_pool(name="gps", bufs=2, space="PSUM") as gps,
            tc.tile_pool(name="scps", bufs=1, space="PSUM") as scps,
            tc.tile_pool(name="fps", bufs=2, space="PSUM") as fps,
            tc.tile_pool(name="attp", bufs=1) as attp,
            tc.tile_pool(name="opool", bufs=1) as opool,
        ):
            xq_sb = consts.tile([D, NH], f32, tag="xq")
            xk_sb = consts.tile([D, N], f32, tag="xk")
            xkT_sb = consts.tile([128, 4, D], f16, tag="xkT")
            wg1T_sb = consts.tile([D, A], f32, tag="wg1T")
            wg2T_sb = consts.tile([D, A], f32, tag="wg2T")
            waT_sb = consts.tile([128, 2], f16, tag="waT")
            bgc_sb = consts.tile([128, 2], f32, tag="bgc")
            sgb_sb = consts.tile([128, 1], f32, tag="sgb")

            nc.sync.dma_start(xq_sb[:], xq.ap())
            nc.sync.dma_start(xk_sb[:], xk.ap())
            nc.sync.dma_start(
                xkT_sb[:], xkT.ap().rearrange("(mb p) d -> p mb d", p=128)
            )
            nc.sync.dma_start(wg1T_sb[:], wg1T.ap())
            nc.sync.dma_start(wg2T_sb[:], wg2T.ap())
            nc.sync.dma_start(waT_sb[:], waT.ap())
            nc.sync.dma_start(bgc_sb[:], bgc.ap())
            nc.sync.dma_start(sgb_sb[:], sgb.ap())

            # g2T[a, m] and g1T[a, n] with bg added (fp32)
            g1b_sb = gbuf.tile([128, 2, NH], f32, tag="g1b")
            g2b_sb = gbuf.tile([128, 2, N], f32, tag="g2b")
            for c in range(2):
                gt2 = gps.tile([128, N], f32, tag="gt")
                nc.tensor.matmul(
                    gt2[:], wg2T_sb[:, c * 128 : (c + 1) * 128], xk_sb[:]
                )
                nc.scalar.activation(
                    g2b_sb[:, c, :], gt2[:], AF.Identity, bias=bgc_sb[:, c : c + 1]
                )
            for c in range(2):
                gt1 = gps.tile([128, N], f32, tag="gt")
                nc.tensor.matmul(
                    gt1[:, :NH], wg1T_sb[:, c * 128 : (c + 1) * 128], xq_sb[:]
                )
                nc.scalar.activation(
                    g1b_sb[:, c, :], gt1[:, :NH], AF.Identity,
                    bias=bgc_sb[:, c : c + 1],
                )

            # scoreT accumulators: 4 m-blocks x [128, NH] fp32 (one bank each)
            sc = [scps.tile([128, NH], f32, tag=f"sc{mb}", name=f"sc{mb}") for mb in range(4)]

            for g in range(NH // G):
                z = zpool.tile([128, G, 2, N], f32, tag="z")
                t = tpool.tile([128, G, 2, N], f16, tag="t")
                for j in range(G):
                    n = g * G + j
                    for c in range(2):
                        nc.vector.tensor_scalar_add(
                            z[:, j, c, :], g2b_sb[:, c, :], g1b_sb[:, c, n : n + 1]
                        )
                nc.scalar.activation(t[:], z[:], AF.Tanh)
                for j in range(G):
                    n = g * G + j
                    for mb in range(4):
                        for c in range(2):
                            nc.tensor.matmul(
                                sc[mb][:, n : n + 1],
                                t[:, j, c, mb * 128 : (mb + 1) * 128],
                                waT_sb[:, c : c + 1],
                                start=(c == 0),
                                stop=(c == 1),
                            )

            # sigmoid -> attT[m, n] fp16
            attT = attp.tile([128, 4, NH], f16, tag="attT")
            for mb in range(4):
                nc.scalar.activation(
                    attT[:, mb, :], sc[mb][:], AF.Sigmoid, bias=sgb_sb[:, 0:1]
                )

            # final: out[n, d] accumulated over m-chunks
            out_sb = opool.tile([128, 2, D], f32, tag="out")
            for nb in range(2):
                fo = fps.tile([128, D], f32, tag="fo")
                for mb in range(4):
                    nc.tensor.matmul(
                        fo[:],
                        attT[:, mb, nb * 128 : (nb + 1) * 128],
                        xkT_sb[:, mb, :],
                        start=(mb == 0),
                        stop=(mb == 3),
                    )
                nc.vector.tensor_copy(out_sb[:, nb, :], fo[:])

            nc.sync.dma_start(
                out.ap().rearrange("(nb p) d -> p nb d", p=128), out_sb[:]
            )

    nc.compile()
    return nc


def _prep_inputs(x, Wg1, Wg2, bg, Wa_w, Wa_b, ba):
    """Build the 8 per-core input maps (host-side slicing/packing only)."""
    x = np.asarray(x, np.float32)
    wg1T = np.ascontiguousarray(np.asarray(Wg1, np.float32).T)   # (96, 256)
    wg2T = np.ascontiguousarray(np.asarray(Wg2, np.float32).T)
    waT = np.ascontiguousarray(
        np.asarray(Wa_w, np.float32).reshape(2, 128).T.astype(np.float16)
    )                                                             # (128, 2)
    bgc = np.ascontiguousarray(np.asarray(bg, np.float32).reshape(2, 128).T)
    sgb = np.full((128, 1), float(np.asarray(Wa_b).ravel()[0])
                  + float(np.asarray(ba).ravel()[0]), np.float32)
    in_maps = []
    for c in range(NCORES):
        b, half = c // 2, c % 2
        xb = x[b]                                                 # (96, 512)
        in_maps.append({
            "xq": np.ascontiguousarray(xb[:, half * NH : (half + 1) * NH]),
            "xk": np.ascontiguousarray(xb),
            "xkT": np.ascontiguousarray(xb.T.astype(np.float16)),
            "wg1T": wg1T,
            "wg2T": wg2T,
            "waT": waT,
            "bgc": bgc,
            "sgb": sgb,
        })
    return in_maps


def _run(inputs, trace=False):
    from concourse.bass_utils import run_bass_kernel_spmd

    if "nc" not in _cache:
        _cache["nc"] = _build_nc()
    nc = _cache["nc"]
    in_maps = _prep_inputs(**inputs)
    res = run_bass_kernel_spmd(
        nc, in_maps, core_ids=list(range(NCORES)), trace=trace
    )
    out = np.empty((B, N, D), np.float32)
    for c in range(NCORES):
        b, half = c // 2, c % 2
        out[b, half * NH : (half + 1) * NH] = res.results[c]["out"]
    return out, res


def kernel(**inputs):
    out, _ = _run(inputs, trace=False)
    return out


# revision 5
# speedup vs baseline: 2.6430x; 2.6430x over previous
"""Additive-attention kernel for Trainium2 (8 NeuronCores, SPMD).

Problem (per batch b of B=4):
    xt      = x[b].T                                  # (N=512, D=96)
    g1      = xt @ Wg1.T                              # (512, 256)
    g2      = xt @ Wg2.T                              # (512, 256)
    score   = sum_a Wa[a] * tanh(g1[n,a] + g2[m,a] + bg[a])    # (512, 512)
    att     = sigmoid(score + Wa_b + ba)
    out[b]  = att @ xt                                # (512, 96)

Sharding: core c handles batch b = c//2 and query-rows n in
[(c%2)*256, (c%2)*256+256).  Each core computes its full out rows;
host concatenates.

Per-core device algorithm ("column scheme"):
  - g1T[a, n] (own n) and g2T[a, m] (all m) via PE matmuls with K=D=96,
    a-chunks of 128 on partitions; bg folded in via ACT per-partition bias.
  - loop over own n: Z[a, m] = g2T[a, m] + g1T[a, n] via DVE tensor_scalar
    (per-partition scalar add), batches of 4 n; one big ACT Tanh
    [128, 4096] fp32->fp16.
  - scoring: per (n, m-block, a-chunk) matmul with the tanh tile as the
    STATIONARY operand ([K=128a, M=128m]) and Wa chunk [128,1] moving ->
    scoreT[m, n] accumulates as dense PSUM columns (4 banks [128, 256]).
  - sigmoid (+Wa_b+ba bias) PSUM->SBUF fp16 gives attT[m, n] directly.
  - final: out[n, d] = sum_m att[n, m] x[b][d, m]: lhsT = attT chunk,
    rhs = xkT chunk (host-passed x[b].T in fp16), accumulate 4 m-chunks.
"""

import numpy as np

B, D, N, A = 4, 96, 512, 256
NH = N // 2          # query rows per core
G = 4                # n-group per tanh op
NCORES = 8

# ── v2: Fourier factorization of the pairwise tanh ──────────────────
# tanh(u+v) ≈ Σ_{j=1..J} BJ[j-1]·sin(j·S·(u+v)), period 2L=32 covers
# |u+v|≤12; coefficients from a smoothness-regularized weighted LS fit
# (free completion on 12<|w|<16).  sin(jθu+jθv) expands into products of
# per-side features sin(jθ)/cos(jθ), built from one ACT Sin pair via the
# Chebyshev recurrence  f_j = 2cosθ·f_{j-1} − f_{j-2}  on DVE (fp16).
FJ = 16
FL = 16.0
FS = float(np.pi / FL)
BJ = [1.75710447, -0.91861438, 1.54626958, -1.24739822, 1.31439916,
      -0.89685277, 0.67809788, -0.27509646, 0.06971438, 0.15176616,
      -0.20417812, 0.23540547, -0.18173202, 0.13053501, -0.06843893,
      0.03608079]

VERSION = 2

_cache = {}


def _build_nc():
    import concourse.bacc as bacc
    import concourse.mybir as mybir
    from concourse import tile

    f32 = mybir.dt.float32
    f16 = mybir.dt.float16
    AF = mybir.ActivationFunctionType

    nc = bacc.Bacc("TRN2", target_bir_lowering=False)

    xq = nc.dram_tensor("xq", [D, NH], f32, kind="ExternalInput")
    xk = nc.dram_tensor("xk", [D, N], f32, kind="ExternalInput")
    xkT = nc.dram_tensor("xkT", [N, D], f16, kind="ExternalInput")
    wg1T = nc.dram_tensor("wg1T", [D, A], f32, kind="ExternalInput")
    wg2T = nc.dram_tensor("wg2T", [D, A], f32, kind="ExternalInput")
    waT = nc.dram_tensor("waT", [128, 2], f16, kind="ExternalInput")
    bgc = nc.dram_tensor("bgc", [128, 2], f32, kind="ExternalInput")
    sgb = nc.dram_tensor("sgb", [128, 1], f32, kind="ExternalInput")
    out = nc.dram_tensor("out", [NH, D], f32, kind="ExternalOutput")

    with tile.TileContext(nc) as tc:
        with (
            tc.tile_pool(name="consts", bufs=1) as consts,
            tc.tile_pool(name="gbuf", bufs=1) as gbuf,
            tc.tile_pool(name="zpool", bufs=2) as zpool,
            tc.tile_pool(name="tpool", bufs=2) as tpool,
            tc.tile_pool(name="gps", bufs=2, space="PSUM") as gps,
            tc.tile_pool(name="scps", bufs=1, space="PSUM") as scps,
            tc.tile_pool(name="fps", bufs=2, space="PSUM") as fps,
            tc.tile_pool(name="attp", bufs=1) as attp,
            tc.tile_pool(name="opool", bufs=1) as opool,
        ):
            xq_sb = consts.tile([D, NH], f32, tag="xq")
            xk_sb = consts.tile([D, N], f32, tag="xk")
            xkT_sb = consts.tile([128, 4, D], f16, tag="xkT")
            wg1T_sb = consts.tile([D, A], f32, tag="wg1T")
            wg2T_sb = consts.tile([D, A], f32, tag="wg2T")
            waT_sb = consts.tile([128, 2], f16, tag="waT")
            bgc_sb = consts.tile([128, 2], f32, tag="bgc")
            sgb_sb = consts.tile([128, 1], f32, tag="sgb")

            nc.sync.dma_start(xq_sb[:], xq.ap())
            nc.sync.dma_start(xk_sb[:], xk.ap())
            nc.sync.dma_start(
                xkT_sb[:], xkT.ap().rearrange("(mb p) d -> p mb d", p=128)
            )
            nc.sync.dma_start(wg1T_sb[:], wg1T.ap())
            nc.sync.dma_start(wg2T_sb[:], wg2T.ap())
            nc.sync.dma_start(waT_sb[:], waT.ap())
            nc.sync.dma_start(bgc_sb[:], bgc.ap())
            nc.sync.dma_start(sgb_sb[:], sgb.ap())

            # g2T[a, m] and g1T[a, n] with bg added (fp32)
            g1b_sb = gbuf.tile([128, 2, NH], f32, tag="g1b")
            g2b_sb = gbuf.tile([128, 2, N], f32, tag="g2b")
            for c in range(2):
                gt2 = gps.tile([128, N], f32, tag="gt")
                nc.tensor.matmul(
                    gt2[:], wg2T_sb[:, c * 128 : (c + 1) * 128], xk_sb[:]
                )
                nc.scalar.activation(
                    g2b_sb[:, c, :], gt2[:], AF.Identity, bias=bgc_sb[:, c : c + 1]
                )
            for c in range(2):
                gt1 = gps.tile([128, N], f32, tag="gt")
                nc.tensor.matmul(
                    gt1[:, :NH], wg1T_sb[:, c * 128 : (c + 1) * 128], xq_sb[:]
                )
                nc.scalar.activation(
                    g1b_sb[:, c, :], gt1[:, :NH], AF.Identity,
                    bias=bgc_sb[:, c : c + 1],
                )

            # scoreT accumulators: 4 m-blocks x [128, NH] fp32 (one bank each)
            sc = [scps.tile([128, NH], f32, tag=f"sc{mb}", name=f"sc{mb}") for mb in range(4)]

            for g in range(NH // G):
                z = zpool.tile([128, G, 2, N], f32, tag="z")
                t = tpool.tile([128, G, 2, N], f16, tag="t")
                for j in range(G):
                    n = g * G + j
                    for c in range(2):
                        nc.vector.tensor_scalar_add(
                            z[:, j, c, :], g2b_sb[:, c, :], g1b_sb[:, c, n : n + 1]
                        )
                nc.scalar.activation(t[:], z[:], AF.Tanh)
                for j in range(G):
                    n = g * G + j
                    for mb in range(4):
                        for c in range(2):
                            nc.tensor.matmul(
                                sc[mb][:, n : n + 1],
                                t[:, j, c, mb * 128 : (mb + 1) * 128],
                                waT_sb[:, c : c + 1],
                                start=(c == 0),
                                stop=(c == 1),
                            )

            # sigmoid -> attT[m, n] fp16
            attT = attp.tile([128, 4, NH], f16, tag="attT")
            for mb in range(4):
                nc.scalar.activation(
                    attT[:, mb, :], sc[mb][:], AF.Sigmoid, bias=sgb_sb[:, 0:1]
                )

            # final: out[n, d] accumulated over m-chunks
            out_sb = opool.tile([128, 2, D], f32, tag="out")
            for nb in range(2):
                fo = fps.tile([128, D], f32, tag="fo")
                for mb in range(4):
                    nc.tensor.matmul(
                        fo[:],
                        attT[:, mb, nb * 128 : (nb + 1) * 128],
                        xkT_sb[:, mb, :],
                        start=(mb == 0),
                        stop=(mb == 3),
                    )
                nc.vector.tensor_copy(out_sb[:, nb, :], fo[:])

            nc.sync.dma_start(
                out.ap().rearrange("(nb p) d -> p nb d", p=128), out_sb[:]
            )

    nc.compile()
    return nc


def _build_nc_v2():
    import concourse.bacc as bacc
    import concourse.mybir as mybir
    from concourse import tile

    f32 = mybir.dt.float32
    f16 = mybir.dt.float16
    AF = mybir.ActivationFunctionType

    nc = bacc.Bacc("TRN2", target_bir_lowering=False)

    xq = nc.dram_tensor("xq", [D, NH], f32, kind="ExternalInput")
    xk = nc.dram_tensor("xk", [D, N], f32, kind="ExternalInput")
    xkT = nc.dram_tensor("xkT", [N, D], f16, kind="ExternalInput")
    w1s = nc.dram_tensor("w1s", [D, A], f32, kind="ExternalInput")   # S*Wg1.T
    w2s = nc.dram_tensor("w2s", [D, A], f32, kind="ExternalInput")   # S*Wg2.T
    bsin = nc.dram_tensor("bsin", [128, 2], f32, kind="ExternalInput")  # S*bg
    bcos = nc.dram_tensor("bcos", [128, 2], f32, kind="ExternalInput")  # S*bg+pi/2
    wav = nc.dram_tensor("wav", [128, 2], f32, kind="ExternalInput")    # Wa chunks
    sgb = nc.dram_tensor("sgb", [128, 1], f32, kind="ExternalInput")
    out = nc.dram_tensor("out", [NH, D], f32, kind="ExternalOutput")

    MULT = mybir.AluOpType.mult

    with tile.TileContext(nc) as tc:
        with (
            tc.tile_pool(name="consts", bufs=1) as consts,
            tc.tile_pool(name="ufeat", bufs=1) as ufeat,
            tc.tile_pool(name="vfeat", bufs=1) as vfeat,
            tc.tile_pool(name="uscal", bufs=1) as uscal,
            tc.tile_pool(name="tmpp", bufs=2) as tmpp,
            tc.tile_pool(name="gps", bufs=2, space="PSUM") as gps,
            tc.tile_pool(name="scps", bufs=1, space="PSUM") as scps,
            tc.tile_pool(name="fps", bufs=1, space="PSUM") as fps,
            tc.tile_pool(name="attp", bufs=1) as attp,
            tc.tile_pool(name="opool", bufs=1) as opool,
        ):
            xq_sb = consts.tile([D, NH], f32, tag="xq")
            xk_sb = consts.tile([D, N], f32, tag="xk")
            xkT_sb = consts.tile([128, 4, D], f16, tag="xkT")
            w1_sb = consts.tile([D, A], f32, tag="w1")
            w2_sb = consts.tile([D, A], f32, tag="w2")
            bsin_sb = consts.tile([128, 2], f32, tag="bsin")
            bcos_sb = consts.tile([128, 2], f32, tag="bcos")
            wav_sb = consts.tile([128, 2], f32, tag="wav")
            sgb_sb = consts.tile([128, 1], f32, tag="sgb")

            nc.sync.dma_start(xq_sb[:], xq.ap())
            nc.sync.dma_start(xk_sb[:], xk.ap())
            nc.sync.dma_start(
                xkT_sb[:], xkT.ap().rearrange("(mb p) d -> p mb d", p=128)
            )
            nc.sync.dma_start(w1_sb[:], w1s.ap())
            nc.sync.dma_start(w2_sb[:], w2s.ap())
            nc.sync.dma_start(bsin_sb[:], bsin.ap())
            nc.sync.dma_start(bcos_sb[:], bcos.ap())
            nc.sync.dma_start(wav_sb[:], wav.ap())
            nc.sync.dma_start(sgb_sb[:], sgb.ap())

            # feature tiles, j = 0..FJ: [128, (sin|cos), chunk*W + col]
            uf = [ufeat.tile([128, 2, NH * 2], f16, tag=f"uf{j}", name=f"uf{j}")
                  for j in range(FJ + 1)]
            vf = [vfeat.tile([128, 2, N * 2], f16, tag=f"vf{j}", name=f"vf{j}")
                  for j in range(FJ + 1)]
            us = [uscal.tile([128, 2, NH * 2], f16, tag=f"us{j}", name=f"us{j}")
                  for j in range(FJ + 1)]
            twou = consts.tile([128, NH * 2], f16, tag="twou")
            twov = consts.tile([128, N * 2], f16, tag="twov")

            # j=0: sin0 = 0, cos0 = 1
            nc.gpsimd.memset(uf[0][:, 0, :], 0.0)
            nc.gpsimd.memset(uf[0][:, 1, :], 1.0)
            nc.gpsimd.memset(vf[0][:, 0, :], 0.0)
            nc.gpsimd.memset(vf[0][:, 1, :], 1.0)

            # theta tiles + base features (j=1)
            for c in range(2):
                thv = gps.tile([128, N], f32, tag="th")
                nc.tensor.matmul(thv[:], w2_sb[:, c * 128:(c + 1) * 128], xk_sb[:])
                nc.scalar.activation(vf[1][:, 0, c * N:(c + 1) * N], thv[:],
                                     AF.Sin, bias=bsin_sb[:, c:c + 1])
                nc.scalar.activation(vf[1][:, 1, c * N:(c + 1) * N], thv[:],
                                     AF.Sin, bias=bcos_sb[:, c:c + 1])
            for c in range(2):
                thu = gps.tile([128, N], f32, tag="th")
                nc.tensor.matmul(thu[:, :NH], w1_sb[:, c * 128:(c + 1) * 128],
                                 xq_sb[:])
                nc.scalar.activation(uf[1][:, 0, c * NH:(c + 1) * NH],
                                     thu[:, :NH], AF.Sin,
                                     bias=bsin_sb[:, c:c + 1])
                nc.scalar.activation(uf[1][:, 1, c * NH:(c + 1) * NH],
                                     thu[:, :NH], AF.Sin,
                                     bias=bcos_sb[:, c:c + 1])

            nc.vector.tensor_scalar_mul(twou[:], uf[1][:, 1, :], 2.0)
            nc.vector.tensor_scalar_mul(twov[:], vf[1][:, 1, :], 2.0)

            sc = [scps.tile([128, NH], f32, tag=f"sc{mb}", name=f"sc{mb}")
                  for mb in range(4)]

            for j in range(1, FJ + 1):
                if j >= 2:
                    tmpu = tmpp.tile([128, 2, NH * 2], f16, tag="tmpu")
                    tmpv = tmpp.tile([128, 2, N * 2], f16, tag="tmpv")
                    for fn in range(2):
                        nc.vector.tensor_mul(tmpu[:, fn, :], uf[j - 1][:, fn, :],
                                             twou[:])
                        nc.vector.tensor_sub(uf[j][:, fn, :], tmpu[:, fn, :],
                                             uf[j - 2][:, fn, :])
                        nc.vector.tensor_mul(tmpv[:, fn, :], vf[j - 1][:, fn, :],
                                             twov[:])
                        nc.vector.tensor_sub(vf[j][:, fn, :], tmpv[:, fn, :],
                                             vf[j - 2][:, fn, :])
                # scale u-features by Wa[a]*BJ[j-1]
                for fn in range(2):
                    for c in range(2):
                        nc.vector.tensor_scalar(
                            us[j][:, fn, c * NH:(c + 1) * NH],
                            uf[j][:, fn, c * NH:(c + 1) * NH],
                            wav_sb[:, c:c + 1], float(BJ[j - 1]),
                            MULT, MULT,
                        )
                # scoring: sin_u pairs cos_v, cos_u pairs sin_v
                for fn in range(2):
                    for c in range(2):
                        for mb in range(4):
                            nc.tensor.matmul(
                                sc[mb][:],
                                vf[j][:, 1 - fn,
                                      c * N + mb * 128: c * N + (mb + 1) * 128],
                                us[j][:, fn, c * NH:(c + 1) * NH],
                                start=(j == 1 and fn == 0 and c == 0),
                                stop=(j == FJ and fn == 1 and c == 1),
                                skip_group_check=True,
                            )

            attT = attp.tile([128, 4, NH], f16, tag="attT")
            for mb in range(4):
                nc.scalar.activation(
                    attT[:, mb, :], sc[mb][:], AF.Sigmoid, bias=sgb_sb[:, 0:1]
                )

            out_sb = opool.tile([128, 2, D], f32, tag="out")
            for nb in range(2):
                fo = fps.tile([128, D], f32, tag="fo")
                for mb in range(4):
                    nc.tensor.matmul(
                        fo[:],
                        attT[:, mb, nb * 128:(nb + 1) * 128],
                        xkT_sb[:, mb, :],
                        start=(mb == 0),
                        stop=(mb == 3),
                    )
                nc.vector.tensor_copy(out_sb[:, nb, :], fo[:])

            nc.sync.dma_start(
                out.ap().rearrange("(nb p) d -> p nb d", p=128), out_sb[:]
            )

    nc.compile()
    return nc


def _prep_inputs_v2(x, Wg1, Wg2, bg, Wa_w, Wa_b, ba):
    x = np.asarray(x, np.float32)
    w1s = np.ascontiguousarray(FS * np.asarray(Wg1, np.float32).T)
    w2s = np.ascontiguousarray(FS * np.asarray(Wg2, np.float32).T)
    bgv = FS * np.asarray(bg, np.float32)
    bsin = np.ascontiguousarray(bgv.reshape(2, 128).T)
    bcos = np.ascontiguousarray((bgv + 0.0).reshape(2, 128).T
                                + np.float32(np.pi / 2))
    wav = np.ascontiguousarray(np.asarray(Wa_w, np.float32).reshape(2, 128).T)
    sgb = np.full((128, 1), float(np.asarray(Wa_b).ravel()[0])
                  + float(np.asarray(ba).ravel()[0]), np.float32)
    in_maps = []
    for c in range(NCORES):
        b, half = c // 2, c % 2
        xb = x[b]
        in_maps.append({
            "xq": np.ascontiguousarray(xb[:, half * NH:(half + 1) * NH]),
            "xk": np.ascontiguousarray(xb),
            "xkT": np.ascontiguousarray(xb.T.astype(np.float16)),
            "w1s": w1s,
            "w2s": w2s,
            "bsin": bsin,
            "bcos": bcos,
            "wav": wav,
            "sgb": sgb,
        })
    return in_maps


def _prep_inputs(x, Wg1, Wg2, bg, Wa_w, Wa_b, ba):
    """Build the 8 per-core input maps (host-side slicing/packing only)."""
    x = np.asarray(x, np.float32)
    wg1T = np.ascontiguousarray(np.asarray(Wg1, np.float32).T)   # (96, 256)
    wg2T = np.ascontiguousarray(np.asarray(Wg2, np.float32).T)
    waT = np.ascontiguousarray(
        np.asarray(Wa_w, np.float32).reshape(2, 128).T.astype(np.float16)
    )                                                             # (128, 2)
    bgc = np.ascontiguousarray(np.asarray(bg, np.float32).reshape(2, 128).T)
    sgb = np.full((128, 1), float(np.asarray(Wa_b).ravel()[0])
                  + float(np.asarray(ba).ravel()[0]), np.float32)
    in_maps = []
    for c in range(NCORES):
        b, half = c // 2, c % 2
        xb = x[b]                                                 # (96, 512)
        in_maps.append({
            "xq": np.ascontiguousarray(xb[:, half * NH : (half + 1) * NH]),
            "xk": np.ascontiguousarray(xb),
            "xkT": np.ascontiguousarray(xb.T.astype(np.float16)),
            "wg1T": wg1T,
            "wg2T": wg2T,
            "waT": waT,
            "bgc": bgc,
            "sgb": sgb,
        })
    return in_maps


def _run(inputs, trace=False):
    from concourse.bass_utils import run_bass_kernel_spmd

    if "nc" not in _cache:
        _cache["nc"] = _build_nc_v2() if VERSION == 2 else _build_nc()
    nc = _cache["nc"]
    in_maps = (_prep_inputs_v2 if VERSION == 2 else _prep_inputs)(**inputs)
    res = run_bass_kernel_spmd(
        nc, in_maps, core_ids=list(range(NCORES)), trace=trace
    )
    out = np.empty((B, N, D), np.float32)
    for c in range(NCORES):
        b, half = c // 2, c % 2
        out[b, half * NH : (half + 1) * NH] = res.results[c]["out"]
    return out, res


def kernel(**inputs):
    out, _ = _run(inputs, trace=False)
    return out


# revision 8
# speedup vs baseline: 2.9827x; 1.1285x over previous
"""Additive-attention kernel for Trainium2 (8 NeuronCores, SPMD).

Problem (per batch b of B=4):
    xt      = x[b].T                                  # (N=512, D=96)
    g1      = xt @ Wg1.T                              # (512, 256)
    g2      = xt @ Wg2.T                              # (512, 256)
    score   = sum_a Wa[a] * tanh(g1[n,a] + g2[m,a] + bg[a])    # (512, 512)
    att     = sigmoid(score + Wa_b + ba)
    out[b]  = att @ xt                                # (512, 96)

Sharding: core c handles batch b = c//2 and query-rows n in
[(c%2)*256, (c%2)*256+256).  Each core computes its full out rows;
host concatenates.

Per-core device algorithm ("column scheme"):
  - g1T[a, n] (own n) and g2T[a, m] (all m) via PE matmuls with K=D=96,
    a-chunks of 128 on partitions; bg folded in via ACT per-partition bias.
  - loop over own n: Z[a, m] = g2T[a, m] + g1T[a, n] via DVE tensor_scalar
    (per-partition scalar add), batches of 4 n; one big ACT Tanh
    [128, 4096] fp32->fp16.
  - scoring: per (n, m-block, a-chunk) matmul with the tanh tile as the
    STATIONARY operand ([K=128a, M=128m]) and Wa chunk [128,1] moving ->
    scoreT[m, n] accumulates as dense PSUM columns (4 banks [128, 256]).
  - sigmoid (+Wa_b+ba bias) PSUM->SBUF fp16 gives attT[m, n] directly.
  - final: out[n, d] = sum_m att[n, m] x[b][d, m]: lhsT = attT chunk,
    rhs = xkT chunk (host-passed x[b].T in fp16), accumulate 4 m-chunks.
"""

import numpy as np

B, D, N, A = 4, 96, 512, 256
NH = N // 2          # query rows per core
G = 4                # n-group per tanh op
NCORES = 8

# ── v2: Fourier factorization of the pairwise tanh ──────────────────
# tanh(u+v) ≈ Σ_{j=1..J} BJ[j-1]·sin(j·S·(u+v)), period 2L=32 covers
# |u+v|≤12; coefficients from a smoothness-regularized weighted LS fit
# (free completion on 12<|w|<16).  sin(jθu+jθv) expands into products of
# per-side features sin(jθ)/cos(jθ), built from one ACT Sin pair via the
# Chebyshev recurrence  f_j = 2cosθ·f_{j-1} − f_{j-2}  on DVE (fp16).
FJ = 16
FL = 16.0
FS = float(np.pi / FL)
BJ = [1.75710447, -0.91861438, 1.54626958, -1.24739822, 1.31439916,
      -0.89685277, 0.67809788, -0.27509646, 0.06971438, 0.15176616,
      -0.20417812, 0.23540547, -0.18173202, 0.13053501, -0.06843893,
      0.03608079]

VERSION = 2

_cache = {}


def _build_nc():
    import concourse.bacc as bacc
    import concourse.mybir as mybir
    from concourse import tile

    f32 = mybir.dt.float32
    f16 = mybir.dt.float16
    AF = mybir.ActivationFunctionType

    nc = bacc.Bacc("TRN2", target_bir_lowering=False)

    xq = nc.dram_tensor("xq", [D, NH], f32, kind="ExternalInput")
    xk = nc.dram_tensor("xk", [D, N], f32, kind="ExternalInput")
    xkT = nc.dram_tensor("xkT", [N, D], f16, kind="ExternalInput")
    wg1T = nc.dram_tensor("wg1T", [D, A], f32, kind="ExternalInput")
    wg2T = nc.dram_tensor("wg2T", [D, A], f32, kind="ExternalInput")
    waT = nc.dram_tensor("waT", [128, 2], f16, kind="ExternalInput")
    bgc = nc.dram_tensor("bgc", [128, 2], f32, kind="ExternalInput")
    sgb = nc.dram_tensor("sgb", [128, 1], f32, kind="ExternalInput")
    out = nc.dram_tensor("out", [NH, D], f32, kind="ExternalOutput")

    with tile.TileContext(nc) as tc:
        with (
            tc.tile_pool(name="consts", bufs=1) as consts,
            tc.tile_pool(name="gbuf", bufs=1) as gbuf,
            tc.tile_pool(name="zpool", bufs=2) as zpool,
            tc.tile_pool(name="tpool", bufs=2) as tpool,
            tc.tile_pool(name="gps", bufs=2, space="PSUM") as gps,
            tc.tile_pool(name="scps", bufs=1, space="PSUM") as scps,
            tc.tile_pool(name="fps", bufs=2, space="PSUM") as fps,
            tc.tile_pool(name="attp", bufs=1) as attp,
            tc.tile_pool(name="opool", bufs=1) as opool,
        ):
            xq_sb = consts.tile([D, NH], f32, tag="xq")
            xk_sb = consts.tile([D, N], f32, tag="xk")
            xkT_sb = consts.tile([128, 4, D], f16, tag="xkT")
            wg1T_sb = consts.tile([D, A], f32, tag="wg1T")
            wg2T_sb = consts.tile([D, A], f32, tag="wg2T")
            waT_sb = consts.tile([128, 2], f16, tag="waT")
            bgc_sb = consts.tile([128, 2], f32, tag="bgc")
            sgb_sb = consts.tile([128, 1], f32, tag="sgb")

            nc.sync.dma_start(xq_sb[:], xq.ap())
            nc.sync.dma_start(xk_sb[:], xk.ap())
            nc.sync.dma_start(
                xkT_sb[:], xkT.ap().rearrange("(mb p) d -> p mb d", p=128)
            )
            nc.sync.dma_start(wg1T_sb[:], wg1T.ap())
            nc.sync.dma_start(wg2T_sb[:], wg2T.ap())
            nc.sync.dma_start(waT_sb[:], waT.ap())
            nc.sync.dma_start(bgc_sb[:], bgc.ap())
            nc.sync.dma_start(sgb_sb[:], sgb.ap())

            # g2T[a, m] and g1T[a, n] with bg added (fp32)
            g1b_sb = gbuf.tile([128, 2, NH], f32, tag="g1b")
            g2b_sb = gbuf.tile([128, 2, N], f32, tag="g2b")
            for c in range(2):
                gt2 = gps.tile([128, N], f32, tag="gt")
                nc.tensor.matmul(
                    gt2[:], wg2T_sb[:, c * 128 : (c + 1) * 128], xk_sb[:]
                )
                nc.scalar.activation(
                    g2b_sb[:, c, :], gt2[:], AF.Identity, bias=bgc_sb[:, c : c + 1]
                )
            for c in range(2):
                gt1 = gps.tile([128, N], f32, tag="gt")
                nc.tensor.matmul(
                    gt1[:, :NH], wg1T_sb[:, c * 128 : (c + 1) * 128], xq_sb[:]
                )
                nc.scalar.activation(
                    g1b_sb[:, c, :], gt1[:, :NH], AF.Identity,
                    bias=bgc_sb[:, c : c + 1],
                )

            # scoreT accumulators: 4 m-blocks x [128, NH] fp32 (one bank each)
            sc = [scps.tile([128, NH], f32, tag=f"sc{mb}", name=f"sc{mb}") for mb in range(4)]

            for g in range(NH // G):
                z = zpool.tile([128, G, 2, N], f32, tag="z")
                t = tpool.tile([128, G, 2, N], f16, tag="t")
                for j in range(G):
                    n = g * G + j
                    for c in range(2):
                        nc.vector.tensor_scalar_add(
                            z[:, j, c, :], g2b_sb[:, c, :], g1b_sb[:, c, n : n + 1]
                        )
                nc.scalar.activation(t[:], z[:], AF.Tanh)
                for j in range(G):
                    n = g * G + j
                    for mb in range(4):
                        for c in range(2):
                            nc.tensor.matmul(
                                sc[mb][:, n : n + 1],
                                t[:, j, c, mb * 128 : (mb + 1) * 128],
                                waT_sb[:, c : c + 1],
                                start=(c == 0),
                                stop=(c == 1),
                            )

            # sigmoid -> attT[m, n] fp16
            attT = attp.tile([128, 4, NH], f16, tag="attT")
            for mb in range(4):
                nc.scalar.activation(
                    attT[:, mb, :], sc[mb][:], AF.Sigmoid, bias=sgb_sb[:, 0:1]
                )

            # final: out[n, d] accumulated over m-chunks
            out_sb = opool.tile([128, 2, D], f32, tag="out")
            for nb in range(2):
                fo = fps.tile([128, D], f32, tag="fo")
                for mb in range(4):
                    nc.tensor.matmul(
                        fo[:],
                        attT[:, mb, nb * 128 : (nb + 1) * 128],
                        xkT_sb[:, mb, :],
                        start=(mb == 0),
                        stop=(mb == 3),
                    )
                nc.vector.tensor_copy(out_sb[:, nb, :], fo[:])

            nc.sync.dma_start(
                out.ap().rearrange("(nb p) d -> p nb d", p=128), out_sb[:]
            )

    nc.compile()
    return nc


def _build_nc_v2():
    import concourse.bacc as bacc
    import concourse.mybir as mybir
    from concourse import tile

    f32 = mybir.dt.float32
    f16 = mybir.dt.float16
    AF = mybir.ActivationFunctionType

    nc = bacc.Bacc("TRN2", target_bir_lowering=False)

    xq = nc.dram_tensor("xq", [D, NH], f32, kind="ExternalInput")
    xk = nc.dram_tensor("xk", [D, N], f32, kind="ExternalInput")
    xkT = nc.dram_tensor("xkT", [N, D], f16, kind="ExternalInput")
    w1s = nc.dram_tensor("w1s", [D, A], f32, kind="ExternalInput")   # S*Wg1.T
    w2s = nc.dram_tensor("w2s", [D, A], f32, kind="ExternalInput")   # S*Wg2.T
    bsin = nc.dram_tensor("bsin", [128, 2], f32, kind="ExternalInput")  # S*bg
    bcos = nc.dram_tensor("bcos", [128, 2], f32, kind="ExternalInput")  # S*bg+pi/2
    wav = nc.dram_tensor("wav", [128, 2], f32, kind="ExternalInput")    # Wa chunks
    sgb = nc.dram_tensor("sgb", [128, 1], f32, kind="ExternalInput")
    out = nc.dram_tensor("out", [NH, D], f32, kind="ExternalOutput")

    MULT = mybir.AluOpType.mult

    with tile.TileContext(nc) as tc:
        with (
            tc.tile_pool(name="consts", bufs=1) as consts,
            tc.tile_pool(name="ufeat", bufs=1) as ufeat,
            tc.tile_pool(name="vfeat", bufs=1) as vfeat,
            tc.tile_pool(name="uscal", bufs=1) as uscal,
            tc.tile_pool(name="tmpp", bufs=2) as tmpp,
            tc.tile_pool(name="gps", bufs=2, space="PSUM") as gps,
            tc.tile_pool(name="scps", bufs=1, space="PSUM") as scps,
            tc.tile_pool(name="fps", bufs=1, space="PSUM") as fps,
            tc.tile_pool(name="attp", bufs=1) as attp,
            tc.tile_pool(name="opool", bufs=1) as opool,
        ):
            xq_sb = consts.tile([D, NH], f32, tag="xq")
            xk_sb = consts.tile([D, N], f32, tag="xk")
            xkT_sb = consts.tile([128, 4, D], f16, tag="xkT")
            w1_sb = consts.tile([D, A], f32, tag="w1")
            w2_sb = consts.tile([D, A], f32, tag="w2")
            bsin_sb = consts.tile([128, 2], f32, tag="bsin")
            bcos_sb = consts.tile([128, 2], f32, tag="bcos")
            wav_sb = consts.tile([128, 2], f32, tag="wav")
            sgb_sb = consts.tile([128, 1], f32, tag="sgb")

            # order: feed the theta matmuls first; xkT only needed at the end
            nc.sync.dma_start(w2_sb[:], w2s.ap())
            nc.sync.dma_start(xk_sb[:], xk.ap())
            nc.sync.dma_start(bsin_sb[:], bsin.ap())
            nc.sync.dma_start(bcos_sb[:], bcos.ap())
            nc.sync.dma_start(w1_sb[:], w1s.ap())
            nc.sync.dma_start(xq_sb[:], xq.ap())
            nc.sync.dma_start(wav_sb[:], wav.ap())
            nc.sync.dma_start(sgb_sb[:], sgb.ap())
            nc.sync.dma_start(
                xkT_sb[:], xkT.ap().rearrange("(mb p) d -> p mb d", p=128)
            )

            # feature tiles, j = 1..FJ: [128, (sin|cos), chunk*W + col]
            uf = [ufeat.tile([128, 2, NH * 2], f16, tag=f"uf{j}", name=f"uf{j}")
                  if j >= 1 else None for j in range(FJ + 1)]
            vf = [vfeat.tile([128, 2, N * 2], f16, tag=f"vf{j}", name=f"vf{j}")
                  if j >= 1 else None for j in range(FJ + 1)]
            us = [uscal.tile([128, 2, NH * 2], f16, tag=f"us{j}", name=f"us{j}")
                  if j >= 1 else None for j in range(FJ + 1)]
            # doubled 2cos(theta) tiles so one DVE op covers both fn halves
            twou = consts.tile([128, 2, NH * 2], f16, tag="twou")
            twov = consts.tile([128, 2, N * 2], f16, tag="twov")

            # theta tiles + base features (j=1)
            for c in range(2):
                thv = gps.tile([128, N], f32, tag="th")
                nc.tensor.matmul(thv[:], w2_sb[:, c * 128:(c + 1) * 128], xk_sb[:])
                nc.scalar.activation(vf[1][:, 0, c * N:(c + 1) * N], thv[:],
                                     AF.Sin, bias=bsin_sb[:, c:c + 1])
                nc.scalar.activation(vf[1][:, 1, c * N:(c + 1) * N], thv[:],
                                     AF.Sin, bias=bcos_sb[:, c:c + 1])
            for c in range(2):
                thu = gps.tile([128, N], f32, tag="th")
                nc.tensor.matmul(thu[:, :NH], w1_sb[:, c * 128:(c + 1) * 128],
                                 xq_sb[:])
                nc.scalar.activation(uf[1][:, 0, c * NH:(c + 1) * NH],
                                     thu[:, :NH], AF.Sin,
                                     bias=bsin_sb[:, c:c + 1])
                nc.scalar.activation(uf[1][:, 1, c * NH:(c + 1) * NH],
                                     thu[:, :NH], AF.Sin,
                                     bias=bcos_sb[:, c:c + 1])

            nc.vector.tensor_scalar_mul(twou[:, 0, :], uf[1][:, 1, :], 2.0)
            nc.vector.tensor_scalar_mul(twou[:, 1, :], uf[1][:, 1, :], 2.0)
            nc.vector.tensor_scalar_mul(twov[:, 0, :], vf[1][:, 1, :], 2.0)
            nc.vector.tensor_scalar_mul(twov[:, 1, :], vf[1][:, 1, :], 2.0)

            sc = [scps.tile([128, NH], f32, tag=f"sc{mb}", name=f"sc{mb}")
                  for mb in range(4)]

            for j in range(1, FJ + 1):
                if j == 2:
                    # f_2 = 2c*f_1 - f_0 with f_0 = (0, 1):
                    # sin_2 = twoc*sin_1;  cos_2 = twoc*cos_1 - 1
                    tmpu = tmpp.tile([128, 2, NH * 2], f16, tag="tmpu")
                    tmpv = tmpp.tile([128, 2, N * 2], f16, tag="tmpv")
                    nc.vector.tensor_mul(tmpu[:], uf[1][:], twou[:])
                    nc.vector.tensor_mul(tmpv[:], vf[1][:], twov[:])
                    nc.vector.tensor_copy(uf[2][:, 0, :], tmpu[:, 0, :])
                    nc.vector.tensor_scalar_add(uf[2][:, 1, :], tmpu[:, 1, :],
                                                -1.0)
                    nc.vector.tensor_copy(vf[2][:, 0, :], tmpv[:, 0, :])
                    nc.vector.tensor_scalar_add(vf[2][:, 1, :], tmpv[:, 1, :],
                                                -1.0)
                elif j >= 3:
                    tmpu = tmpp.tile([128, 2, NH * 2], f16, tag="tmpu")
                    tmpv = tmpp.tile([128, 2, N * 2], f16, tag="tmpv")
                    nc.vector.tensor_mul(tmpu[:], uf[j - 1][:], twou[:])
                    nc.vector.tensor_sub(uf[j][:], tmpu[:], uf[j - 2][:])
                    nc.vector.tensor_mul(tmpv[:], vf[j - 1][:], twov[:])
                    nc.vector.tensor_sub(vf[j][:], tmpv[:], vf[j - 2][:])
                # scale u-features by Wa[a]*BJ[j-1] (both fn halves per op)
                for c in range(2):
                    nc.vector.tensor_scalar(
                        us[j][:, :, c * NH:(c + 1) * NH],
                        uf[j][:, :, c * NH:(c + 1) * NH],
                        wav_sb[:, c:c + 1], float(BJ[j - 1]),
                        MULT, MULT,
                    )
                # scoring: sin_u pairs cos_v, cos_u pairs sin_v
                for fn in range(2):
                    for c in range(2):
                        for mb in range(4):
                            nc.tensor.matmul(
                                sc[mb][:],
                                vf[j][:, 1 - fn,
                                      c * N + mb * 128: c * N + (mb + 1) * 128],
                                us[j][:, fn, c * NH:(c + 1) * NH],
                                start=(j == 1 and fn == 0 and c == 0),
                                stop=(j == FJ and fn == 1 and c == 1),
                                skip_group_check=True,
                            )

            attT = attp.tile([128, 4, NH], f16, tag="attT")
            for mb in range(4):
                nc.scalar.activation(
                    attT[:, mb, :], sc[mb][:], AF.Sigmoid, bias=sgb_sb[:, 0:1]
                )

            out_sb = opool.tile([128, 2, D], f32, tag="out")
            for nb in range(2):
                fo = fps.tile([128, D], f32, tag="fo")
                for mb in range(4):
                    nc.tensor.matmul(
                        fo[:],
                        attT[:, mb, nb * 128:(nb + 1) * 128],
                        xkT_sb[:, mb, :],
                        start=(mb == 0),
                        stop=(mb == 3),
                    )
                nc.vector.tensor_copy(out_sb[:, nb, :], fo[:])

            nc.sync.dma_start(
                out.ap().rearrange("(nb p) d -> p nb d", p=128), out_sb[:]
            )

    nc.compile()
    return nc


def _prep_inputs_v2(x, Wg1, Wg2, bg, Wa_w, Wa_b, ba):
    x = np.asarray(x, np.float32)
    w1s = np.ascontiguousarray(FS * np.asarray(Wg1, np.float32).T)
    w2s = np.ascontiguousarray(FS * np.asarray(Wg2, np.float32).T)
    bgv = FS * np.asarray(bg, np.float32)
    bsin = np.ascontiguousarray(bgv.reshape(2, 128).T)
    bcos = np.ascontiguousarray((bgv + 0.0).reshape(2, 128).T
                                + np.float32(np.pi / 2))
    wav = np.ascontiguousarray(np.asarray(Wa_w, np.float32).reshape(2, 128).T)
    sgb = np.full((128, 1), float(np.asarray(Wa_b).ravel()[0])
                  + float(np.asarray(ba).ravel()[0]), np.float32)
    in_maps = []
    for c in range(NCORES):
        b, half = c // 2, c % 2
        xb = x[b]
        in_maps.append({
            "xq": np.ascontiguousarray(xb[:, half * NH:(half + 1) * NH]),
            "xk": np.ascontiguousarray(xb),
            "xkT": np.ascontiguousarray(xb.T.astype(np.float16)),
            "w1s": w1s,
            "w2s": w2s,
            "bsin": bsin,
            "bcos": bcos,
            "wav": wav,
            "sgb": sgb,
        })
    return in_maps


def _prep_inputs(x, Wg1, Wg2, bg, Wa_w, Wa_b, ba):
    """Build the 8 per-core input maps (host-side slicing/packing only)."""
    x = np.asarray(x, np.float32)
    wg1T = np.ascontiguousarray(np.asarray(Wg1, np.float32).T)   # (96, 256)
    wg2T = np.ascontiguousarray(np.asarray(Wg2, np.float32).T)
    waT = np.ascontiguousarray(
        np.asarray(Wa_w, np.float32).reshape(2, 128).T.astype(np.float16)
    )                                                             # (128, 2)
    bgc = np.ascontiguousarray(np.asarray(bg, np.float32).reshape(2, 128).T)
    sgb = np.full((128, 1), float(np.asarray(Wa_b).ravel()[0])
                  + float(np.asarray(ba).ravel()[0]), np.float32)
    in_maps = []
    for c in range(NCORES):
        b, half = c // 2, c % 2
        xb = x[b]                                                 # (96, 512)
        in_maps.append({
            "xq": np.ascontiguousarray(xb[:, half * NH : (half + 1) * NH]),
            "xk": np.ascontiguousarray(xb),
            "xkT": np.ascontiguousarray(xb.T.astype(np.float16)),
            "wg1T": wg1T,
            "wg2T": wg2T,
            "waT": waT,
            "bgc": bgc,
            "sgb": sgb,
        })
    return in_maps


def _run(inputs, trace=False):
    from concourse.bass_utils import run_bass_kernel_spmd

    if "nc" not in _cache:
        _cache["nc"] = _build_nc_v2() if VERSION == 2 else _build_nc()
    nc = _cache["nc"]
    in_maps = (_prep_inputs_v2 if VERSION == 2 else _prep_inputs)(**inputs)
    res = run_bass_kernel_spmd(
        nc, in_maps, core_ids=list(range(NCORES)), trace=trace
    )
    out = np.empty((B, N, D), np.float32)
    for c in range(NCORES):
        b, half = c // 2, c % 2
        out[b, half * NH : (half + 1) * NH] = res.results[c]["out"]
    return out, res


def kernel(**inputs):
    out, _ = _run(inputs, trace=False)
    return out


# revision 12
# speedup vs baseline: 3.0084x; 1.0086x over previous
"""Additive-attention kernel for Trainium2 (8 NeuronCores, SPMD).

Problem (per batch b of B=4):
    xt      = x[b].T                                  # (N=512, D=96)
    g1      = xt @ Wg1.T                              # (512, 256)
    g2      = xt @ Wg2.T                              # (512, 256)
    score   = sum_a Wa[a] * tanh(g1[n,a] + g2[m,a] + bg[a])    # (512, 512)
    att     = sigmoid(score + Wa_b + ba)
    out[b]  = att @ xt                                # (512, 96)

Sharding: core c handles batch b = c//2 and query-rows n in
[(c%2)*256, (c%2)*256+256).  Each core computes its full out rows;
host concatenates.

Per-core device algorithm ("column scheme"):
  - g1T[a, n] (own n) and g2T[a, m] (all m) via PE matmuls with K=D=96,
    a-chunks of 128 on partitions; bg folded in via ACT per-partition bias.
  - loop over own n: Z[a, m] = g2T[a, m] + g1T[a, n] via DVE tensor_scalar
    (per-partition scalar add), batches of 4 n; one big ACT Tanh
    [128, 4096] fp32->fp16.
  - scoring: per (n, m-block, a-chunk) matmul with the tanh tile as the
    STATIONARY operand ([K=128a, M=128m]) and Wa chunk [128,1] moving ->
    scoreT[m, n] accumulates as dense PSUM columns (4 banks [128, 256]).
  - sigmoid (+Wa_b+ba bias) PSUM->SBUF fp16 gives attT[m, n] directly.
  - final: out[n, d] = sum_m att[n, m] x[b][d, m]: lhsT = attT chunk,
    rhs = xkT chunk (host-passed x[b].T in fp16), accumulate 4 m-chunks.
"""

import numpy as np

B, D, N, A = 4, 96, 512, 256
NH = N // 2          # query rows per core
G = 4                # n-group per tanh op
NCORES = 8

# ── v2: Fourier factorization of the pairwise tanh ──────────────────
# tanh(u+v) ≈ Σ_{j=1..J} BJ[j-1]·sin(j·S·(u+v)), period 2L=32 covers
# |u+v|≤12; coefficients from a smoothness-regularized weighted LS fit
# (free completion on 12<|w|<16).  sin(jθu+jθv) expands into products of
# per-side features sin(jθ)/cos(jθ), built from one ACT Sin pair via the
# Chebyshev recurrence  f_j = 2cosθ·f_{j-1} − f_{j-2}  on DVE (fp16).
FJ = 16
FL = 16.0
FS = float(np.pi / FL)
BJ = [1.75710447, -0.91861438, 1.54626958, -1.24739822, 1.31439916,
      -0.89685277, 0.67809788, -0.27509646, 0.06971438, 0.15176616,
      -0.20417812, 0.23540547, -0.18173202, 0.13053501, -0.06843893,
      0.03608079]

VERSION = 2

_cache = {}


def _build_nc():
    import concourse.bacc as bacc
    import concourse.mybir as mybir
    from concourse import tile

    f32 = mybir.dt.float32
    f16 = mybir.dt.float16
    AF = mybir.ActivationFunctionType

    nc = bacc.Bacc("TRN2", target_bir_lowering=False)

    xq = nc.dram_tensor("xq", [D, NH], f32, kind="ExternalInput")
    xk = nc.dram_tensor("xk", [D, N], f32, kind="ExternalInput")
    xkT = nc.dram_tensor("xkT", [N, D], f16, kind="ExternalInput")
    wg1T = nc.dram_tensor("wg1T", [D, A], f32, kind="ExternalInput")
    wg2T = nc.dram_tensor("wg2T", [D, A], f32, kind="ExternalInput")
    waT = nc.dram_tensor("waT", [128, 2], f16, kind="ExternalInput")
    bgc = nc.dram_tensor("bgc", [128, 2], f32, kind="ExternalInput")
    sgb = nc.dram_tensor("sgb", [128, 1], f32, kind="ExternalInput")
    out = nc.dram_tensor("out", [NH, D], f32, kind="ExternalOutput")

    with tile.TileContext(nc) as tc:
        with (
            tc.tile_pool(name="consts", bufs=1) as consts,
            tc.tile_pool(name="gbuf", bufs=1) as gbuf,
            tc.tile_pool(name="zpool", bufs=2) as zpool,
            tc.tile_pool(name="tpool", bufs=2) as tpool,
            tc.tile_pool(name="gps", bufs=2, space="PSUM") as gps,
            tc.tile_pool(name="scps", bufs=1, space="PSUM") as scps,
            tc.tile_pool(name="fps", bufs=2, space="PSUM") as fps,
            tc.tile_pool(name="attp", bufs=1) as attp,
            tc.tile_pool(name="opool", bufs=1) as opool,
        ):
            xq_sb = consts.tile([D, NH], f32, tag="xq")
            xk_sb = consts.tile([D, N], f32, tag="xk")
            xkT_sb = consts.tile([128, 4, D], f16, tag="xkT")
            wg1T_sb = consts.tile([D, A], f32, tag="wg1T")
            wg2T_sb = consts.tile([D, A], f32, tag="wg2T")
            waT_sb = consts.tile([128, 2], f16, tag="waT")
            bgc_sb = consts.tile([128, 2], f32, tag="bgc")
            sgb_sb = consts.tile([128, 1], f32, tag="sgb")

            nc.sync.dma_start(xq_sb[:], xq.ap())
            nc.sync.dma_start(xk_sb[:], xk.ap())
            nc.sync.dma_start(
                xkT_sb[:], xkT.ap().rearrange("(mb p) d -> p mb d", p=128)
            )
            nc.sync.dma_start(wg1T_sb[:], wg1T.ap())
            nc.sync.dma_start(wg2T_sb[:], wg2T.ap())
            nc.sync.dma_start(waT_sb[:], waT.ap())
            nc.sync.dma_start(bgc_sb[:], bgc.ap())
            nc.sync.dma_start(sgb_sb[:], sgb.ap())

            # g2T[a, m] and g1T[a, n] with bg added (fp32)
            g1b_sb = gbuf.tile([128, 2, NH], f32, tag="g1b")
            g2b_sb = gbuf.tile([128, 2, N], f32, tag="g2b")
            for c in range(2):
                gt2 = gps.tile([128, N], f32, tag="gt")
                nc.tensor.matmul(
                    gt2[:], wg2T_sb[:, c * 128 : (c + 1) * 128], xk_sb[:]
                )
                nc.scalar.activation(
                    g2b_sb[:, c, :], gt2[:], AF.Identity, bias=bgc_sb[:, c : c + 1]
                )
            for c in range(2):
                gt1 = gps.tile([128, N], f32, tag="gt")
                nc.tensor.matmul(
                    gt1[:, :NH], wg1T_sb[:, c * 128 : (c + 1) * 128], xq_sb[:]
                )
                nc.scalar.activation(
                    g1b_sb[:, c, :], gt1[:, :NH], AF.Identity,
                    bias=bgc_sb[:, c : c + 1],
                )

            # scoreT accumulators: 4 m-blocks x [128, NH] fp32 (one bank each)
            sc = [scps.tile([128, NH], f32, tag=f"sc{mb}", name=f"sc{mb}") for mb in range(4)]

            for g in range(NH // G):
                z = zpool.tile([128, G, 2, N], f32, tag="z")
                t = tpool.tile([128, G, 2, N], f16, tag="t")
                for j in range(G):
                    n = g * G + j
                    for c in range(2):
                        nc.vector.tensor_scalar_add(
                            z[:, j, c, :], g2b_sb[:, c, :], g1b_sb[:, c, n : n + 1]
                        )
                nc.scalar.activation(t[:], z[:], AF.Tanh)
                for j in range(G):
                    n = g * G + j
                    for mb in range(4):
                        for c in range(2):
                            nc.tensor.matmul(
                                sc[mb][:, n : n + 1],
                                t[:, j, c, mb * 128 : (mb + 1) * 128],
                                waT_sb[:, c : c + 1],
                                start=(c == 0),
                                stop=(c == 1),
                            )

            # sigmoid -> attT[m, n] fp16
            attT = attp.tile([128, 4, NH], f16, tag="attT")
            for mb in range(4):
                nc.scalar.activation(
                    attT[:, mb, :], sc[mb][:], AF.Sigmoid, bias=sgb_sb[:, 0:1]
                )

            # final: out[n, d] accumulated over m-chunks
            out_sb = opool.tile([128, 2, D], f32, tag="out")
            for nb in range(2):
                fo = fps.tile([128, D], f32, tag="fo")
                for mb in range(4):
                    nc.tensor.matmul(
                        fo[:],
                        attT[:, mb, nb * 128 : (nb + 1) * 128],
                        xkT_sb[:, mb, :],
                        start=(mb == 0),
                        stop=(mb == 3),
                    )
                nc.vector.tensor_copy(out_sb[:, nb, :], fo[:])

            nc.sync.dma_start(
                out.ap().rearrange("(nb p) d -> p nb d", p=128), out_sb[:]
            )

    nc.compile()
    return nc


def _build_nc_v2():
    import concourse.bacc as bacc
    import concourse.mybir as mybir
    from concourse import tile

    f32 = mybir.dt.float32
    f16 = mybir.dt.float16
    AF = mybir.ActivationFunctionType

    nc = bacc.Bacc("TRN2", target_bir_lowering=False)

    xq = nc.dram_tensor("xq", [D, NH], f32, kind="ExternalInput")
    xk = nc.dram_tensor("xk", [D, N], f32, kind="ExternalInput")
    xkT = nc.dram_tensor("xkT", [N, D], f16, kind="ExternalInput")
    w1s = nc.dram_tensor("w1s", [D, A], f32, kind="ExternalInput")   # S*Wg1.T
    w2s = nc.dram_tensor("w2s", [D, A], f32, kind="ExternalInput")   # S*Wg2.T
    bsin = nc.dram_tensor("bsin", [128, 2], f32, kind="ExternalInput")  # S*bg
    bcos = nc.dram_tensor("bcos", [128, 2], f32, kind="ExternalInput")  # S*bg+pi/2
    wav = nc.dram_tensor("wav", [128, 2], f32, kind="ExternalInput")    # Wa chunks
    sgb = nc.dram_tensor("sgb", [128, 1], f32, kind="ExternalInput")
    out = nc.dram_tensor("out", [NH, D], f32, kind="ExternalOutput")

    MULT = mybir.AluOpType.mult

    with tile.TileContext(nc) as tc:
        with (
            tc.tile_pool(name="consts", bufs=1) as consts,
            tc.tile_pool(name="ufeat", bufs=1) as ufeat,
            tc.tile_pool(name="vfeat", bufs=1) as vfeat,
            tc.tile_pool(name="uscal", bufs=1) as uscal,
            tc.tile_pool(name="tmpp", bufs=2) as tmpp,
            tc.tile_pool(name="gps", bufs=2, space="PSUM") as gps,
            tc.tile_pool(name="scps", bufs=1, space="PSUM") as scps,
            tc.tile_pool(name="fps", bufs=1, space="PSUM") as fps,
            tc.tile_pool(name="attp", bufs=1) as attp,
            tc.tile_pool(name="opool", bufs=1) as opool,
        ):
            xq_sb = consts.tile([D, NH], f32, tag="xq")
            xk_sb = consts.tile([D, N], f32, tag="xk")
            xkT_sb = consts.tile([128, 4, D], f16, tag="xkT")
            w1_sb = consts.tile([D, A], f32, tag="w1")
            w2_sb = consts.tile([D, A], f32, tag="w2")
            bsin_sb = consts.tile([128, 2], f32, tag="bsin")
            bcos_sb = consts.tile([128, 2], f32, tag="bcos")
            wav_sb = consts.tile([128, 2], f32, tag="wav")
            sgb_sb = consts.tile([128, 1], f32, tag="sgb")

            # dummy Sin on garbage to preload the ACT table set during DMAs
            dummy = consts.tile([128, 1], f32, tag="dummy")
            nc.gpsimd.memset(dummy[:], 0.0)
            nc.scalar.activation(dummy[:], dummy[:], AF.Sin)

            # critical-path DMAs on the sync queue; bulky xkT on the
            # scalar queue (only needed at the very end)
            nc.sync.dma_start(w2_sb[:], w2s.ap())
            nc.sync.dma_start(xk_sb[:], xk.ap())
            nc.sync.dma_start(bsin_sb[:], bsin.ap())
            nc.sync.dma_start(bcos_sb[:], bcos.ap())
            nc.sync.dma_start(w1_sb[:], w1s.ap())
            nc.sync.dma_start(xq_sb[:], xq.ap())
            nc.sync.dma_start(wav_sb[:], wav.ap())
            nc.sync.dma_start(sgb_sb[:], sgb.ap())
            nc.scalar.dma_start(
                xkT_sb[:], xkT.ap().rearrange("(mb p) d -> p mb d", p=128)
            )

            # feature tiles, j = 1..FJ: [128, (sin|cos), chunk*W + col]
            uf = [ufeat.tile([128, 2, NH * 2], f16, tag=f"uf{j}", name=f"uf{j}")
                  if j >= 1 else None for j in range(FJ + 1)]
            vf = [vfeat.tile([128, 2, N * 2], f16, tag=f"vf{j}", name=f"vf{j}")
                  if j >= 1 else None for j in range(FJ + 1)]
            us = [uscal.tile([128, 2, NH * 2], f16, tag=f"us{j}", name=f"us{j}")
                  if j >= 1 else None for j in range(FJ + 1)]
            # doubled 2cos(theta) tiles so one DVE op covers both fn halves
            twou = consts.tile([128, 2, NH * 2], f16, tag="twou")
            twov = consts.tile([128, 2, N * 2], f16, tag="twov")

            # theta tiles + base features (j=1); cos first (twoc needs it)
            thvs = []
            for c in range(2):
                thv = gps.tile([128, N], f32, tag="th", name=f"thv{c}")
                nc.tensor.matmul(thv[:], w2_sb[:, c * 128:(c + 1) * 128], xk_sb[:])
                thvs.append(thv)
            thus = []
            for c in range(2):
                thu = gps.tile([128, N], f32, tag="th", name=f"thu{c}")
                nc.tensor.matmul(thu[:, :NH], w1_sb[:, c * 128:(c + 1) * 128],
                                 xq_sb[:])
                thus.append(thu)
            # v-side first (longer recurrence side), per-tile cos then sin
            for c in range(2):
                nc.scalar.activation(vf[1][:, 1, c * N:(c + 1) * N], thvs[c][:],
                                     AF.Sin, bias=bcos_sb[:, c:c + 1])
                nc.scalar.activation(vf[1][:, 0, c * N:(c + 1) * N], thvs[c][:],
                                     AF.Sin, bias=bsin_sb[:, c:c + 1])
            for c in range(2):
                nc.scalar.activation(uf[1][:, 1, c * NH:(c + 1) * NH],
                                     thus[c][:, :NH], AF.Sin,
                                     bias=bcos_sb[:, c:c + 1])
                nc.scalar.activation(uf[1][:, 0, c * NH:(c + 1) * NH],
                                     thus[c][:, :NH], AF.Sin,
                                     bias=bsin_sb[:, c:c + 1])

            nc.vector.tensor_scalar_mul(twov[:, 0, :], vf[1][:, 1, :], 2.0)
            nc.vector.tensor_scalar_mul(twov[:, 1, :], vf[1][:, 1, :], 2.0)
            nc.vector.tensor_scalar_mul(twou[:, 0, :], uf[1][:, 1, :], 2.0)
            nc.vector.tensor_scalar_mul(twou[:, 1, :], uf[1][:, 1, :], 2.0)

            sc = [scps.tile([128, NH], f32, tag=f"sc{mb}", name=f"sc{mb}")
                  for mb in range(4)]

            for j in range(1, FJ + 1):
                if j == 2:
                    # f_2 = 2c*f_1 - f_0 with f_0 = (0, 1):
                    # sin_2 = twoc*sin_1;  cos_2 = twoc*cos_1 - 1
                    tmpu = tmpp.tile([128, 2, NH * 2], f16, tag="tmpu")
                    tmpv = tmpp.tile([128, 2, N * 2], f16, tag="tmpv")
                    nc.vector.tensor_mul(tmpu[:], uf[1][:], twou[:])
                    nc.vector.tensor_mul(tmpv[:], vf[1][:], twov[:])
                    nc.vector.tensor_copy(uf[2][:, 0, :], tmpu[:, 0, :])
                    nc.vector.tensor_scalar_add(uf[2][:, 1, :], tmpu[:, 1, :],
                                                -1.0)
                    nc.vector.tensor_copy(vf[2][:, 0, :], tmpv[:, 0, :])
                    nc.vector.tensor_scalar_add(vf[2][:, 1, :], tmpv[:, 1, :],
                                                -1.0)
                elif j >= 3:
                    tmpu = tmpp.tile([128, 2, NH * 2], f16, tag="tmpu")
                    tmpv = tmpp.tile([128, 2, N * 2], f16, tag="tmpv")
                    nc.vector.tensor_mul(tmpu[:], uf[j - 1][:], twou[:])
                    nc.vector.tensor_sub(uf[j][:], tmpu[:], uf[j - 2][:])
                    nc.vector.tensor_mul(tmpv[:], vf[j - 1][:], twov[:])
                    nc.vector.tensor_sub(vf[j][:], tmpv[:], vf[j - 2][:])
                # scale u-features by Wa[a]*BJ[j-1] (both fn halves per op)
                for c in range(2):
                    nc.vector.tensor_scalar(
                        us[j][:, :, c * NH:(c + 1) * NH],
                        uf[j][:, :, c * NH:(c + 1) * NH],
                        wav_sb[:, c:c + 1], float(BJ[j - 1]),
                        MULT, MULT,
                    )
                # scoring: sin_u pairs cos_v, cos_u pairs sin_v
                for fn in range(2):
                    for c in range(2):
                        for mb in range(4):
                            nc.tensor.matmul(
                                sc[mb][:],
                                vf[j][:, 1 - fn,
                                      c * N + mb * 128: c * N + (mb + 1) * 128],
                                us[j][:, fn, c * NH:(c + 1) * NH],
                                start=(j == 1 and fn == 0 and c == 0),
                                stop=(j == FJ and fn == 1 and c == 1),
                                skip_group_check=True,
                            )

            attT = attp.tile([128, 4, NH], f16, tag="attT")
            out_sb = opool.tile([128, 2, D], f32, tag="out")
            fos = [fps.tile([128, D], f32, tag=f"fo{nb}", name=f"fo{nb}")
                   for nb in range(2)]
            for mb in range(4):
                nc.scalar.activation(
                    attT[:, mb, :], sc[mb][:], AF.Sigmoid, bias=sgb_sb[:, 0:1]
                )
                for nb in range(2):
                    nc.tensor.matmul(
                        fos[nb][:],
                        attT[:, mb, nb * 128:(nb + 1) * 128],
                        xkT_sb[:, mb, :],
                        start=(mb == 0),
                        stop=(mb == 3),
                        skip_group_check=True,
                    )
            for nb in range(2):
                nc.vector.tensor_copy(out_sb[:, nb, :], fos[nb][:])

            nc.sync.dma_start(
                out.ap().rearrange("(nb p) d -> p nb d", p=128), out_sb[:]
            )

    nc.compile()
    return nc


def _prep_inputs_v2(x, Wg1, Wg2, bg, Wa_w, Wa_b, ba):
    x = np.asarray(x, np.float32)
    w1s = np.ascontiguousarray(FS * np.asarray(Wg1, np.float32).T)
    w2s = np.ascontiguousarray(FS * np.asarray(Wg2, np.float32).T)
    bgv = FS * np.asarray(bg, np.float32)
    bsin = np.ascontiguousarray(bgv.reshape(2, 128).T)
    bcos = np.ascontiguousarray((bgv + 0.0).reshape(2, 128).T
                                + np.float32(np.pi / 2))
    wav = np.ascontiguousarray(np.asarray(Wa_w, np.float32).reshape(2, 128).T)
    sgb = np.full((128, 1), float(np.asarray(Wa_b).ravel()[0])
                  + float(np.asarray(ba).ravel()[0]), np.float32)
    in_maps = []
    for c in range(NCORES):
        b, half = c // 2, c % 2
        xb = x[b]
        in_maps.append({
            "xq": np.ascontiguousarray(xb[:, half * NH:(half + 1) * NH]),
            "xk": np.ascontiguousarray(xb),
            "xkT": np.ascontiguousarray(xb.T.astype(np.float16)),
            "w1s": w1s,
            "w2s": w2s,
            "bsin": bsin,
            "bcos": bcos,
            "wav": wav,
            "sgb": sgb,
        })
    return in_maps


def _prep_inputs(x, Wg1, Wg2, bg, Wa_w, Wa_b, ba):
    """Build the 8 per-core input maps (host-side slicing/packing only)."""
    x = np.asarray(x, np.float32)
    wg1T = np.ascontiguousarray(np.asarray(Wg1, np.float32).T)   # (96, 256)
    wg2T = np.ascontiguousarray(np.asarray(Wg2, np.float32).T)
    waT = np.ascontiguousarray(
        np.asarray(Wa_w, np.float32).reshape(2, 128).T.astype(np.float16)
    )                                                             # (128, 2)
    bgc = np.ascontiguousarray(np.asarray(bg, np.float32).reshape(2, 128).T)
    sgb = np.full((128, 1), float(np.asarray(Wa_b).ravel()[0])
                  + float(np.asarray(ba).ravel()[0]), np.float32)
    in_maps = []
    for c in range(NCORES):
        b, half = c // 2, c % 2
        xb = x[b]                                                 # (96, 512)
        in_maps.append({
            "xq": np.ascontiguousarray(xb[:, half * NH : (half + 1) * NH]),
            "xk": np.ascontiguousarray(xb),
            "xkT": np.ascontiguousarray(xb.T.astype(np.float16)),
            "wg1T": wg1T,
            "wg2T": wg2T,
            "waT": waT,
            "bgc": bgc,
            "sgb": sgb,
        })
    return in_maps


def _run(inputs, trace=False):
    from concourse.bass_utils import run_bass_kernel_spmd

    if "nc" not in _cache:
        _cache["nc"] = _build_nc_v2() if VERSION == 2 else _build_nc()
    nc = _cache["nc"]
    in_maps = (_prep_inputs_v2 if VERSION == 2 else _prep_inputs)(**inputs)
    res = run_bass_kernel_spmd(
        nc, in_maps, core_ids=list(range(NCORES)), trace=trace
    )
    out = np.empty((B, N, D), np.float32)
    for c in range(NCORES):
        b, half = c // 2, c % 2
        out[b, half * NH : (half + 1) * NH] = res.results[c]["out"]
    return out, res


def kernel(**inputs):
    out, _ = _run(inputs, trace=False)
    return out


# revision 16
# speedup vs baseline: 3.1564x; 1.0492x over previous
"""Additive-attention kernel for Trainium2 (8 NeuronCores, SPMD).

Problem (per batch b of B=4):
    xt      = x[b].T                                  # (N=512, D=96)
    g1      = xt @ Wg1.T                              # (512, 256)
    g2      = xt @ Wg2.T                              # (512, 256)
    score   = sum_a Wa[a] * tanh(g1[n,a] + g2[m,a] + bg[a])    # (512, 512)
    att     = sigmoid(score + Wa_b + ba)
    out[b]  = att @ xt                                # (512, 96)

Sharding: core c handles batch b = c//2 and query-rows n in
[(c%2)*256, (c%2)*256+256).  Each core computes its full out rows;
host concatenates.

Per-core device algorithm ("column scheme"):
  - g1T[a, n] (own n) and g2T[a, m] (all m) via PE matmuls with K=D=96,
    a-chunks of 128 on partitions; bg folded in via ACT per-partition bias.
  - loop over own n: Z[a, m] = g2T[a, m] + g1T[a, n] via DVE tensor_scalar
    (per-partition scalar add), batches of 4 n; one big ACT Tanh
    [128, 4096] fp32->fp16.
  - scoring: per (n, m-block, a-chunk) matmul with the tanh tile as the
    STATIONARY operand ([K=128a, M=128m]) and Wa chunk [128,1] moving ->
    scoreT[m, n] accumulates as dense PSUM columns (4 banks [128, 256]).
  - sigmoid (+Wa_b+ba bias) PSUM->SBUF fp16 gives attT[m, n] directly.
  - final: out[n, d] = sum_m att[n, m] x[b][d, m]: lhsT = attT chunk,
    rhs = xkT chunk (host-passed x[b].T in fp16), accumulate 4 m-chunks.
"""

import numpy as np

B, D, N, A = 4, 96, 512, 256
NH = N // 2          # query rows per core
G = 4                # n-group per tanh op
NCORES = 8

# ── v2: Fourier factorization of the pairwise tanh ──────────────────
# tanh(u+v) ≈ Σ_{j=1..J} BJ[j-1]·sin(j·S·(u+v)), period 2L=32 covers
# |u+v|≤12; coefficients from a smoothness-regularized weighted LS fit
# (free completion on 12<|w|<16).  sin(jθu+jθv) expands into products of
# per-side features sin(jθ)/cos(jθ), built from one ACT Sin pair via the
# Chebyshev recurrence  f_j = 2cosθ·f_{j-1} − f_{j-2}  on DVE (fp16).
FJ = 13
FL = 13.0
FS = float(np.pi / FL)
BJ = [1.25339337, -0.01945643, 0.37026378, -0.0301986, 0.17909742,
      -0.02951455, 0.0971231, -0.02324765, 0.05208288, -0.01869683,
      0.02430917, -0.01051296, 0.01973076]

VERSION = 2

_cache = {}


def _build_nc():
    import concourse.bacc as bacc
    import concourse.mybir as mybir
    from concourse import tile

    f32 = mybir.dt.float32
    f16 = mybir.dt.float16
    AF = mybir.ActivationFunctionType

    nc = bacc.Bacc("TRN2", target_bir_lowering=False)

    xq = nc.dram_tensor("xq", [D, NH], f32, kind="ExternalInput")
    xk = nc.dram_tensor("xk", [D, N], f32, kind="ExternalInput")
    xkT = nc.dram_tensor("xkT", [N, D], f16, kind="ExternalInput")
    wg1T = nc.dram_tensor("wg1T", [D, A], f32, kind="ExternalInput")
    wg2T = nc.dram_tensor("wg2T", [D, A], f32, kind="ExternalInput")
    waT = nc.dram_tensor("waT", [128, 2], f16, kind="ExternalInput")
    bgc = nc.dram_tensor("bgc", [128, 2], f32, kind="ExternalInput")
    sgb = nc.dram_tensor("sgb", [128, 1], f32, kind="ExternalInput")
    out = nc.dram_tensor("out", [NH, D], f32, kind="ExternalOutput")

    with tile.TileContext(nc) as tc:
        with (
            tc.tile_pool(name="consts", bufs=1) as consts,
            tc.tile_pool(name="gbuf", bufs=1) as gbuf,
            tc.tile_pool(name="zpool", bufs=2) as zpool,
            tc.tile_pool(name="tpool", bufs=2) as tpool,
            tc.tile_pool(name="gps", bufs=2, space="PSUM") as gps,
            tc.tile_pool(name="scps", bufs=1, space="PSUM") as scps,
            tc.tile_pool(name="fps", bufs=2, space="PSUM") as fps,
            tc.tile_pool(name="attp", bufs=1) as attp,
            tc.tile_pool(name="opool", bufs=1) as opool,
        ):
            xq_sb = consts.tile([D, NH], f32, tag="xq")
            xk_sb = consts.tile([D, N], f32, tag="xk")
            xkT_sb = consts.tile([128, 4, D], f16, tag="xkT")
            wg1T_sb = consts.tile([D, A], f32, tag="wg1T")
            wg2T_sb = consts.tile([D, A], f32, tag="wg2T")
            waT_sb = consts.tile([128, 2], f16, tag="waT")
            bgc_sb = consts.tile([128, 2], f32, tag="bgc")
            sgb_sb = consts.tile([128, 1], f32, tag="sgb")

            nc.sync.dma_start(xq_sb[:], xq.ap())
            nc.sync.dma_start(xk_sb[:], xk.ap())
            nc.sync.dma_start(
                xkT_sb[:], xkT.ap().rearrange("(mb p) d -> p mb d", p=128)
            )
            nc.sync.dma_start(wg1T_sb[:], wg1T.ap())
            nc.sync.dma_start(wg2T_sb[:], wg2T.ap())
            nc.sync.dma_start(waT_sb[:], waT.ap())
            nc.sync.dma_start(bgc_sb[:], bgc.ap())
            nc.sync.dma_start(sgb_sb[:], sgb.ap())

            # g2T[a, m] and g1T[a, n] with bg added (fp32)
            g1b_sb = gbuf.tile([128, 2, NH], f32, tag="g1b")
            g2b_sb = gbuf.tile([128, 2, N], f32, tag="g2b")
            for c in range(2):
                gt2 = gps.tile([128, N], f32, tag="gt")
                nc.tensor.matmul(
                    gt2[:], wg2T_sb[:, c * 128 : (c + 1) * 128], xk_sb[:]
                )
                nc.scalar.activation(
                    g2b_sb[:, c, :], gt2[:], AF.Identity, bias=bgc_sb[:, c : c + 1]
                )
            for c in range(2):
                gt1 = gps.tile([128, N], f32, tag="gt")
                nc.tensor.matmul(
                    gt1[:, :NH], wg1T_sb[:, c * 128 : (c + 1) * 128], xq_sb[:]
                )
                nc.scalar.activation(
                    g1b_sb[:, c, :], gt1[:, :NH], AF.Identity,
                    bias=bgc_sb[:, c : c + 1],
                )

            # scoreT accumulators: 4 m-blocks x [128, NH] fp32 (one bank each)
            sc = [scps.tile([128, NH], f32, tag=f"sc{mb}", name=f"sc{mb}") for mb in range(4)]

            for g in range(NH // G):
                z = zpool.tile([128, G, 2, N], f32, tag="z")
                t = tpool.tile([128, G, 2, N], f16, tag="t")
                for j in range(G):
                    n = g * G + j
                    for c in range(2):
                        nc.vector.tensor_scalar_add(
                            z[:, j, c, :], g2b_sb[:, c, :], g1b_sb[:, c, n : n + 1]
                        )
                nc.scalar.activation(t[:], z[:], AF.Tanh)
                for j in range(G):
                    n = g * G + j
                    for mb in range(4):
                        for c in range(2):
                            nc.tensor.matmul(
                                sc[mb][:, n : n + 1],
                                t[:, j, c, mb * 128 : (mb + 1) * 128],
                                waT_sb[:, c : c + 1],
                                start=(c == 0),
                                stop=(c == 1),
                            )

            # sigmoid -> attT[m, n] fp16
            attT = attp.tile([128, 4, NH], f16, tag="attT")
            for mb in range(4):
                nc.scalar.activation(
                    attT[:, mb, :], sc[mb][:], AF.Sigmoid, bias=sgb_sb[:, 0:1]
                )

            # final: out[n, d] accumulated over m-chunks
            out_sb = opool.tile([128, 2, D], f32, tag="out")
            for nb in range(2):
                fo = fps.tile([128, D], f32, tag="fo")
                for mb in range(4):
                    nc.tensor.matmul(
                        fo[:],
                        attT[:, mb, nb * 128 : (nb + 1) * 128],
                        xkT_sb[:, mb, :],
                        start=(mb == 0),
                        stop=(mb == 3),
                    )
                nc.vector.tensor_copy(out_sb[:, nb, :], fo[:])

            nc.sync.dma_start(
                out.ap().rearrange("(nb p) d -> p nb d", p=128), out_sb[:]
            )

    nc.compile()
    return nc


def _build_nc_v2():
    import concourse.bacc as bacc
    import concourse.mybir as mybir
    from concourse import tile

    f32 = mybir.dt.float32
    f16 = mybir.dt.float16
    AF = mybir.ActivationFunctionType

    nc = bacc.Bacc("TRN2", target_bir_lowering=False)

    # packed inputs: one DMA per logical group
    vin = nc.dram_tensor("vin", [D, A + N], f16, kind="ExternalInput")   # S*Wg2.T | x[b]
    uin = nc.dram_tensor("uin", [D, A + NH], f16, kind="ExternalInput")  # S*Wg1.T | xq
    biasv = nc.dram_tensor("biasv", [128, 7], f32, kind="ExternalInput")
    xkT = nc.dram_tensor("xkT", [N, D], f16, kind="ExternalInput")
    out = nc.dram_tensor("out", [NH, D], f32, kind="ExternalOutput")

    MULT = mybir.AluOpType.mult

    with tile.TileContext(nc) as tc:
        with (
            tc.tile_pool(name="consts", bufs=1) as consts,
            tc.tile_pool(name="ufeat", bufs=1) as ufeat,
            tc.tile_pool(name="vfeat", bufs=1) as vfeat,
            tc.tile_pool(name="uscal", bufs=1) as uscal,
            tc.tile_pool(name="tmpp", bufs=2) as tmpp,
            tc.tile_pool(name="gps", bufs=2, space="PSUM") as gps,
            tc.tile_pool(name="scps", bufs=1, space="PSUM") as scps,
            tc.tile_pool(name="fps", bufs=1, space="PSUM") as fps,
            tc.tile_pool(name="attp", bufs=1) as attp,
            tc.tile_pool(name="opool", bufs=1) as opool,
        ):
            vin_sb = consts.tile([D, A + N], f16, tag="vin")
            uin_sb = consts.tile([D, A + NH], f16, tag="uin")
            biasv_sb = consts.tile([128, 7], f32, tag="biasv")
            xkT_sb = consts.tile([128, 4, D], f16, tag="xkT")
            w2_sb = vin_sb[:, :A]
            xk_sb = vin_sb[:, A:A + N]
            w1_sb = uin_sb[:, :A]
            xq_sb = uin_sb[:, A:A + NH]
            bsin_sb = biasv_sb[:, 0:2]
            bcos_sb = biasv_sb[:, 2:4]
            wav_sb = biasv_sb[:, 4:6]
            sgb_sb = biasv_sb[:, 6:7]

            # dummy Sin on garbage to preload the ACT table set during DMAs
            dummy = consts.tile([128, 1], f32, tag="dummy")
            nc.gpsimd.memset(dummy[:], 0.0)
            nc.scalar.activation(dummy[:], dummy[:], AF.Sin)

            # critical-path DMAs on the sync queue; bulky xkT on the
            # scalar queue (only needed at the very end)
            nc.sync.dma_start(vin_sb[:], vin.ap())
            nc.sync.dma_start(biasv_sb[:], biasv.ap())
            nc.sync.dma_start(uin_sb[:], uin.ap())
            nc.scalar.dma_start(
                xkT_sb[:], xkT.ap().rearrange("(mb p) d -> p mb d", p=128)
            )

            # feature tiles, j = 1..FJ: [128, (sin|cos), chunk*W + col]
            uf = [ufeat.tile([128, 2, NH * 2], f16, tag=f"uf{j}", name=f"uf{j}")
                  if j >= 1 else None for j in range(FJ + 1)]
            vf = [vfeat.tile([128, 2, N * 2], f16, tag=f"vf{j}", name=f"vf{j}")
                  if j >= 1 else None for j in range(FJ + 1)]
            us = [uscal.tile([128, 2, NH * 2], f16, tag=f"us{j}", name=f"us{j}")
                  if j >= 1 else None for j in range(FJ + 1)]
            # doubled 2cos(theta) tiles so one DVE op covers both fn halves
            twou = consts.tile([128, 2, NH * 2], f16, tag="twou")
            twov = consts.tile([128, 2, N * 2], f16, tag="twov")

            # theta tiles + base features (j=1); cos first (twoc needs it)
            thvs = []
            for c in range(2):
                thv = gps.tile([128, N], f32, tag="th", name=f"thv{c}")
                nc.tensor.matmul(thv[:], w2_sb[:, c * 128:(c + 1) * 128], xk_sb[:])
                thvs.append(thv)
            thus = []
            for c in range(2):
                thu = gps.tile([128, N], f32, tag="th", name=f"thu{c}")
                nc.tensor.matmul(thu[:, :NH], w1_sb[:, c * 128:(c + 1) * 128],
                                 xq_sb[:])
                thus.append(thu)
            # v-side first (longer recurrence side), per-tile cos then sin
            for c in range(2):
                nc.scalar.activation(vf[1][:, 1, c * N:(c + 1) * N], thvs[c][:],
                                     AF.Sin, bias=bcos_sb[:, c:c + 1])
                nc.scalar.activation(vf[1][:, 0, c * N:(c + 1) * N], thvs[c][:],
                                     AF.Sin, bias=bsin_sb[:, c:c + 1])
            for c in range(2):
                nc.scalar.activation(uf[1][:, 1, c * NH:(c + 1) * NH],
                                     thus[c][:, :NH], AF.Sin,
                                     bias=bcos_sb[:, c:c + 1])
                nc.scalar.activation(uf[1][:, 0, c * NH:(c + 1) * NH],
                                     thus[c][:, :NH], AF.Sin,
                                     bias=bsin_sb[:, c:c + 1])

            nc.vector.tensor_scalar_mul(twov[:, 0, :], vf[1][:, 1, :], 2.0)
            nc.vector.tensor_scalar_mul(twov[:, 1, :], vf[1][:, 1, :], 2.0)
            nc.vector.tensor_scalar_mul(twou[:, 0, :], uf[1][:, 1, :], 2.0)
            nc.vector.tensor_scalar_mul(twou[:, 1, :], uf[1][:, 1, :], 2.0)

            sc = [scps.tile([128, NH], f32, tag=f"sc{mb}", name=f"sc{mb}")
                  for mb in range(4)]

            for j in range(1, FJ + 1):
                if j == 2:
                    # f_2 = 2c*f_1 - f_0 with f_0 = (0, 1):
                    # sin_2 = twoc*sin_1;  cos_2 = twoc*cos_1 - 1
                    tmpu = tmpp.tile([128, 2, NH * 2], f16, tag="tmpu")
                    tmpv = tmpp.tile([128, 2, N * 2], f16, tag="tmpv")
                    nc.vector.tensor_mul(tmpu[:], uf[1][:], twou[:])
                    nc.vector.tensor_mul(tmpv[:], vf[1][:], twov[:])
                    nc.vector.tensor_copy(uf[2][:, 0, :], tmpu[:, 0, :])
                    nc.vector.tensor_scalar_add(uf[2][:, 1, :], tmpu[:, 1, :],
                                                -1.0)
                    nc.vector.tensor_copy(vf[2][:, 0, :], tmpv[:, 0, :])
                    nc.vector.tensor_scalar_add(vf[2][:, 1, :], tmpv[:, 1, :],
                                                -1.0)
                elif j >= 3:
                    tmpu = tmpp.tile([128, 2, NH * 2], f16, tag="tmpu")
                    tmpv = tmpp.tile([128, 2, N * 2], f16, tag="tmpv")
                    nc.vector.tensor_mul(tmpu[:], uf[j - 1][:], twou[:])
                    nc.vector.tensor_sub(uf[j][:], tmpu[:], uf[j - 2][:])
                    nc.vector.tensor_mul(tmpv[:], vf[j - 1][:], twov[:])
                    nc.vector.tensor_sub(vf[j][:], tmpv[:], vf[j - 2][:])
                # scale u-features by Wa[a]*BJ[j-1] (both fn halves per op)
                for c in range(2):
                    nc.vector.tensor_scalar(
                        us[j][:, :, c * NH:(c + 1) * NH],
                        uf[j][:, :, c * NH:(c + 1) * NH],
                        wav_sb[:, c:c + 1], float(BJ[j - 1]),
                        MULT, MULT,
                    )
                # scoring: sin_u pairs cos_v, cos_u pairs sin_v
                for fn in range(2):
                    for c in range(2):
                        for mb in range(4):
                            nc.tensor.matmul(
                                sc[mb][:],
                                vf[j][:, 1 - fn,
                                      c * N + mb * 128: c * N + (mb + 1) * 128],
                                us[j][:, fn, c * NH:(c + 1) * NH],
                                start=(j == 1 and fn == 0 and c == 0),
                                stop=(j == FJ and fn == 1 and c == 1),
                                skip_group_check=True,
                            )

            attT = attp.tile([128, 4, NH], f16, tag="attT")
            out_sb = opool.tile([128, 2, D], f32, tag="out")
            fos = [fps.tile([128, D], f32, tag=f"fo{nb}", name=f"fo{nb}")
                   for nb in range(2)]
            for mb in range(4):
                nc.scalar.activation(
                    attT[:, mb, :], sc[mb][:], AF.Sigmoid, bias=sgb_sb[:, 0:1]
                )
                for nb in range(2):
                    nc.tensor.matmul(
                        fos[nb][:],
                        attT[:, mb, nb * 128:(nb + 1) * 128],
                        xkT_sb[:, mb, :],
                        start=(mb == 0),
                        stop=(mb == 3),
                        skip_group_check=True,
                    )
            for nb in range(2):
                nc.vector.tensor_copy(out_sb[:, nb, :], fos[nb][:])

            nc.sync.dma_start(
                out.ap().rearrange("(nb p) d -> p nb d", p=128), out_sb[:]
            )

    nc.compile()
    return nc


def _prep_inputs_v2(x, Wg1, Wg2, bg, Wa_w, Wa_b, ba):
    x = np.asarray(x, np.float32)
    w1s = (FS * np.asarray(Wg1, np.float32).T).astype(np.float16)
    w2s = (FS * np.asarray(Wg2, np.float32).T).astype(np.float16)
    bgv = FS * np.asarray(bg, np.float32)
    biasv = np.empty((128, 7), np.float32)
    biasv[:, 0:2] = bgv.reshape(2, 128).T
    biasv[:, 2:4] = bgv.reshape(2, 128).T + np.float32(np.pi / 2)
    biasv[:, 4:6] = np.asarray(Wa_w, np.float32).reshape(2, 128).T
    biasv[:, 6] = float(np.asarray(Wa_b).ravel()[0]) \
        + float(np.asarray(ba).ravel()[0])
    in_maps = []
    for c in range(NCORES):
        b, half = c // 2, c % 2
        xb = x[b]
        xb16 = xb.astype(np.float16)
        vin = np.ascontiguousarray(np.concatenate([w2s, xb16], axis=1))
        uin = np.ascontiguousarray(np.concatenate(
            [w1s, xb16[:, half * NH:(half + 1) * NH]], axis=1))
        in_maps.append({
            "vin": vin,
            "uin": uin,
            "biasv": np.ascontiguousarray(biasv),
            "xkT": np.ascontiguousarray(xb.T.astype(np.float16)),
        })
    return in_maps


def _prep_inputs(x, Wg1, Wg2, bg, Wa_w, Wa_b, ba):
    """Build the 8 per-core input maps (host-side slicing/packing only)."""
    x = np.asarray(x, np.float32)
    wg1T = np.ascontiguousarray(np.asarray(Wg1, np.float32).T)   # (96, 256)
    wg2T = np.ascontiguousarray(np.asarray(Wg2, np.float32).T)
    waT = np.ascontiguousarray(
        np.asarray(Wa_w, np.float32).reshape(2, 128).T.astype(np.float16)
    )                                                             # (128, 2)
    bgc = np.ascontiguousarray(np.asarray(bg, np.float32).reshape(2, 128).T)
    sgb = np.full((128, 1), float(np.asarray(Wa_b).ravel()[0])
                  + float(np.asarray(ba).ravel()[0]), np.float32)
    in_maps = []
    for c in range(NCORES):
        b, half = c // 2, c % 2
        xb = x[b]                                                 # (96, 512)
        in_maps.append({
            "xq": np.ascontiguousarray(xb[:, half * NH : (half + 1) * NH]),
            "xk": np.ascontiguousarray(xb),
            "xkT": np.ascontiguousarray(xb.T.astype(np.float16)),
            "wg1T": wg1T,
            "wg2T": wg2T,
            "waT": waT,
            "bgc": bgc,
            "sgb": sgb,
        })
    return in_maps


def _run(inputs, trace=False):
    from concourse.bass_utils import run_bass_kernel_spmd

    if "nc" not in _cache:
        _cache["nc"] = _build_nc_v2() if VERSION == 2 else _build_nc()
    nc = _cache["nc"]
    in_maps = (_prep_inputs_v2 if VERSION == 2 else _prep_inputs)(**inputs)
    res = run_bass_kernel_spmd(
        nc, in_maps, core_ids=list(range(NCORES)), trace=trace
    )
    out = np.empty((B, N, D), np.float32)
    for c in range(NCORES):
        b, half = c // 2, c % 2
        out[b, half * NH : (half + 1) * NH] = res.results[c]["out"]
    return out, res


def kernel(**inputs):
    out, _ = _run(inputs, trace=False)
    return out


# revision 17
# speedup vs baseline: 3.4758x; 1.1012x over previous
"""Additive-attention kernel for Trainium2 (8 NeuronCores, SPMD).

Problem (per batch b of B=4):
    xt      = x[b].T                                  # (N=512, D=96)
    g1      = xt @ Wg1.T                              # (512, 256)
    g2      = xt @ Wg2.T                              # (512, 256)
    score   = sum_a Wa[a] * tanh(g1[n,a] + g2[m,a] + bg[a])    # (512, 512)
    att     = sigmoid(score + Wa_b + ba)
    out[b]  = att @ xt                                # (512, 96)

Sharding: core c handles batch b = c//2 and query-rows n in
[(c%2)*256, (c%2)*256+256).  Each core computes its full out rows;
host concatenates.

Per-core device algorithm ("column scheme"):
  - g1T[a, n] (own n) and g2T[a, m] (all m) via PE matmuls with K=D=96,
    a-chunks of 128 on partitions; bg folded in via ACT per-partition bias.
  - loop over own n: Z[a, m] = g2T[a, m] + g1T[a, n] via DVE tensor_scalar
    (per-partition scalar add), batches of 4 n; one big ACT Tanh
    [128, 4096] fp32->fp16.
  - scoring: per (n, m-block, a-chunk) matmul with the tanh tile as the
    STATIONARY operand ([K=128a, M=128m]) and Wa chunk [128,1] moving ->
    scoreT[m, n] accumulates as dense PSUM columns (4 banks [128, 256]).
  - sigmoid (+Wa_b+ba bias) PSUM->SBUF fp16 gives attT[m, n] directly.
  - final: out[n, d] = sum_m att[n, m] x[b][d, m]: lhsT = attT chunk,
    rhs = xkT chunk (host-passed x[b].T in fp16), accumulate 4 m-chunks.
"""

import numpy as np

B, D, N, A = 4, 96, 512, 256
NH = N // 2          # query rows per core
G = 4                # n-group per tanh op
NCORES = 8

# ── v2: Fourier factorization of the pairwise tanh ──────────────────
# tanh(u+v) ≈ Σ_{j=1..J} BJ[j-1]·sin(j·S·(u+v)), period 2L=32 covers
# |u+v|≤12; coefficients from a smoothness-regularized weighted LS fit
# (free completion on 12<|w|<16).  sin(jθu+jθv) expands into products of
# per-side features sin(jθ)/cos(jθ), built from one ACT Sin pair via the
# Chebyshev recurrence  f_j = 2cosθ·f_{j-1} − f_{j-2}  on DVE (fp16).
FJ = 13
FL = 13.0
FS = float(np.pi / FL)
BJ = [1.25339337, -0.01945643, 0.37026378, -0.0301986, 0.17909742,
      -0.02951455, 0.0971231, -0.02324765, 0.05208288, -0.01869683,
      0.02430917, -0.01051296, 0.01973076]

VERSION = 2

_cache = {}


def _build_nc():
    import concourse.bacc as bacc
    import concourse.mybir as mybir
    from concourse import tile

    f32 = mybir.dt.float32
    f16 = mybir.dt.float16
    AF = mybir.ActivationFunctionType

    nc = bacc.Bacc("TRN2", target_bir_lowering=False)

    xq = nc.dram_tensor("xq", [D, NH], f32, kind="ExternalInput")
    xk = nc.dram_tensor("xk", [D, N], f32, kind="ExternalInput")
    xkT = nc.dram_tensor("xkT", [N, D], f16, kind="ExternalInput")
    wg1T = nc.dram_tensor("wg1T", [D, A], f32, kind="ExternalInput")
    wg2T = nc.dram_tensor("wg2T", [D, A], f32, kind="ExternalInput")
    waT = nc.dram_tensor("waT", [128, 2], f16, kind="ExternalInput")
    bgc = nc.dram_tensor("bgc", [128, 2], f32, kind="ExternalInput")
    sgb = nc.dram_tensor("sgb", [128, 1], f32, kind="ExternalInput")
    out = nc.dram_tensor("out", [NH, D], f32, kind="ExternalOutput")

    with tile.TileContext(nc) as tc:
        with (
            tc.tile_pool(name="consts", bufs=1) as consts,
            tc.tile_pool(name="gbuf", bufs=1) as gbuf,
            tc.tile_pool(name="zpool", bufs=2) as zpool,
            tc.tile_pool(name="tpool", bufs=2) as tpool,
            tc.tile_pool(name="gps", bufs=2, space="PSUM") as gps,
            tc.tile_pool(name="scps", bufs=1, space="PSUM") as scps,
            tc.tile_pool(name="fps", bufs=2, space="PSUM") as fps,
            tc.tile_pool(name="attp", bufs=1) as attp,
            tc.tile_pool(name="opool", bufs=1) as opool,
        ):
            xq_sb = consts.tile([D, NH], f32, tag="xq")
            xk_sb = consts.tile([D, N], f32, tag="xk")
            xkT_sb = consts.tile([128, 4, D], f16, tag="xkT")
            wg1T_sb = consts.tile([D, A], f32, tag="wg1T")
            wg2T_sb = consts.tile([D, A], f32, tag="wg2T")
            waT_sb = consts.tile([128, 2], f16, tag="waT")
            bgc_sb = consts.tile([128, 2], f32, tag="bgc")
            sgb_sb = consts.tile([128, 1], f32, tag="sgb")

            nc.sync.dma_start(xq_sb[:], xq.ap())
            nc.sync.dma_start(xk_sb[:], xk.ap())
            nc.sync.dma_start(
                xkT_sb[:], xkT.ap().rearrange("(mb p) d -> p mb d", p=128)
            )
            nc.sync.dma_start(wg1T_sb[:], wg1T.ap())
            nc.sync.dma_start(wg2T_sb[:], wg2T.ap())
            nc.sync.dma_start(waT_sb[:], waT.ap())
            nc.sync.dma_start(bgc_sb[:], bgc.ap())
            nc.sync.dma_start(sgb_sb[:], sgb.ap())

            # g2T[a, m] and g1T[a, n] with bg added (fp32)
            g1b_sb = gbuf.tile([128, 2, NH], f32, tag="g1b")
            g2b_sb = gbuf.tile([128, 2, N], f32, tag="g2b")
            for c in range(2):
                gt2 = gps.tile([128, N], f32, tag="gt")
                nc.tensor.matmul(
                    gt2[:], wg2T_sb[:, c * 128 : (c + 1) * 128], xk_sb[:]
                )
                nc.scalar.activation(
                    g2b_sb[:, c, :], gt2[:], AF.Identity, bias=bgc_sb[:, c : c + 1]
                )
            for c in range(2):
                gt1 = gps.tile([128, N], f32, tag="gt")
                nc.tensor.matmul(
                    gt1[:, :NH], wg1T_sb[:, c * 128 : (c + 1) * 128], xq_sb[:]
                )
                nc.scalar.activation(
                    g1b_sb[:, c, :], gt1[:, :NH], AF.Identity,
                    bias=bgc_sb[:, c : c + 1],
                )

            # scoreT accumulators: 4 m-blocks x [128, NH] fp32 (one bank each)
            sc = [scps.tile([128, NH], f32, tag=f"sc{mb}", name=f"sc{mb}") for mb in range(4)]

            for g in range(NH // G):
                z = zpool.tile([128, G, 2, N], f32, tag="z")
                t = tpool.tile([128, G, 2, N], f16, tag="t")
                for j in range(G):
                    n = g * G + j
                    for c in range(2):
                        nc.vector.tensor_scalar_add(
                            z[:, j, c, :], g2b_sb[:, c, :], g1b_sb[:, c, n : n + 1]
                        )
                nc.scalar.activation(t[:], z[:], AF.Tanh)
                for j in range(G):
                    n = g * G + j
                    for mb in range(4):
                        for c in range(2):
                            nc.tensor.matmul(
                                sc[mb][:, n : n + 1],
                                t[:, j, c, mb * 128 : (mb + 1) * 128],
                                waT_sb[:, c : c + 1],
                                start=(c == 0),
                                stop=(c == 1),
                            )

            # sigmoid -> attT[m, n] fp16
            attT = attp.tile([128, 4, NH], f16, tag="attT")
            for mb in range(4):
                nc.scalar.activation(
                    attT[:, mb, :], sc[mb][:], AF.Sigmoid, bias=sgb_sb[:, 0:1]
                )

            # final: out[n, d] accumulated over m-chunks
            out_sb = opool.tile([128, 2, D], f32, tag="out")
            for nb in range(2):
                fo = fps.tile([128, D], f32, tag="fo")
                for mb in range(4):
                    nc.tensor.matmul(
                        fo[:],
                        attT[:, mb, nb * 128 : (nb + 1) * 128],
                        xkT_sb[:, mb, :],
                        start=(mb == 0),
                        stop=(mb == 3),
                    )
                nc.vector.tensor_copy(out_sb[:, nb, :], fo[:])

            nc.sync.dma_start(
                out.ap().rearrange("(nb p) d -> p nb d", p=128), out_sb[:]
            )

    nc.compile()
    return nc


def _build_nc_v2():
    import os
    _F32IN = bool(int(os.environ.get("K_F32IN", "0")))
    import concourse.bacc as bacc
    import concourse.mybir as mybir
    from concourse import tile

    f32 = mybir.dt.float32
    f16 = mybir.dt.float16
    AF = mybir.ActivationFunctionType

    nc = bacc.Bacc("TRN2", target_bir_lowering=False)

    # packed inputs: one DMA per logical group
    vin = nc.dram_tensor("vin", [D, A + N], f32 if _F32IN else f16, kind="ExternalInput")   # S*Wg2.T | x[b]
    uin = nc.dram_tensor("uin", [D, A + NH], f32 if _F32IN else f16, kind="ExternalInput")  # S*Wg1.T | xq
    biasv = nc.dram_tensor("biasv", [128, 7], f32, kind="ExternalInput")
    xkT = nc.dram_tensor("xkT", [N, D], f16, kind="ExternalInput")
    out = nc.dram_tensor("out", [NH, D], f32, kind="ExternalOutput")

    MULT = mybir.AluOpType.mult

    with tile.TileContext(nc) as tc:
        with (
            tc.tile_pool(name="consts", bufs=1) as consts,
            tc.tile_pool(name="ufeat", bufs=1) as ufeat,
            tc.tile_pool(name="vfeat", bufs=1) as vfeat,
            tc.tile_pool(name="uscal", bufs=1) as uscal,
            tc.tile_pool(name="tmpp", bufs=2) as tmpp,
            tc.tile_pool(name="gps", bufs=2, space="PSUM") as gps,
            tc.tile_pool(name="scps", bufs=1, space="PSUM") as scps,
            tc.tile_pool(name="fps", bufs=1, space="PSUM") as fps,
            tc.tile_pool(name="attp", bufs=1) as attp,
            tc.tile_pool(name="opool", bufs=1) as opool,
        ):
            vin_sb = consts.tile([D, A + N], f32 if _F32IN else f16, tag="vin")
            uin_sb = consts.tile([D, A + NH], f32 if _F32IN else f16, tag="uin")
            biasv_sb = consts.tile([128, 7], f32, tag="biasv")
            xkT_sb = consts.tile([128, 4, D], f16, tag="xkT")
            w2_sb = vin_sb[:, :A]
            xk_sb = vin_sb[:, A:A + N]
            w1_sb = uin_sb[:, :A]
            xq_sb = uin_sb[:, A:A + NH]
            bsin_sb = biasv_sb[:, 0:2]
            bcos_sb = biasv_sb[:, 2:4]
            wav_sb = biasv_sb[:, 4:6]
            sgb_sb = biasv_sb[:, 6:7]

            # dummy Sin on garbage to preload the ACT table set during DMAs
            dummy = consts.tile([128, 1], f32, tag="dummy")
            nc.gpsimd.memset(dummy[:], 0.0)
            nc.scalar.activation(dummy[:], dummy[:], AF.Sin)

            # critical-path DMAs on the sync queue; bulky xkT on the
            # scalar queue (only needed at the very end)
            nc.sync.dma_start(vin_sb[:], vin.ap())
            nc.sync.dma_start(biasv_sb[:], biasv.ap())
            nc.sync.dma_start(uin_sb[:], uin.ap())
            nc.scalar.dma_start(
                xkT_sb[:], xkT.ap().rearrange("(mb p) d -> p mb d", p=128)
            )

            # feature tiles, j = 1..FJ: [128, (sin|cos), chunk*W + col]
            uf = [ufeat.tile([128, 2, NH * 2], f16, tag=f"uf{j}", name=f"uf{j}")
                  if j >= 1 else None for j in range(FJ + 1)]
            vf = [vfeat.tile([128, 2, N * 2], f16, tag=f"vf{j}", name=f"vf{j}")
                  if j >= 1 else None for j in range(FJ + 1)]
            us = [uscal.tile([128, 2, NH * 2], f16, tag=f"us{j}", name=f"us{j}")
                  if j >= 1 else None for j in range(FJ + 1)]
            # doubled 2cos(theta) tiles so one DVE op covers both fn halves
            twou = consts.tile([128, 2, NH * 2], f16, tag="twou")
            twov = consts.tile([128, 2, N * 2], f16, tag="twov")

            # theta tiles + base features (j=1); cos first (twoc needs it)
            thvs = []
            for c in range(2):
                thv = gps.tile([128, N], f32, tag="th", name=f"thv{c}")
                nc.tensor.matmul(thv[:], w2_sb[:, c * 128:(c + 1) * 128], xk_sb[:])
                thvs.append(thv)
            thus = []
            for c in range(2):
                thu = gps.tile([128, N], f32, tag="th", name=f"thu{c}")
                nc.tensor.matmul(thu[:, :NH], w1_sb[:, c * 128:(c + 1) * 128],
                                 xq_sb[:])
                thus.append(thu)
            # v-side first (longer recurrence side), per-tile cos then sin
            for c in range(2):
                nc.scalar.activation(vf[1][:, 1, c * N:(c + 1) * N], thvs[c][:],
                                     AF.Sin, bias=bcos_sb[:, c:c + 1])
                nc.scalar.activation(vf[1][:, 0, c * N:(c + 1) * N], thvs[c][:],
                                     AF.Sin, bias=bsin_sb[:, c:c + 1])
            for c in range(2):
                nc.scalar.activation(uf[1][:, 1, c * NH:(c + 1) * NH],
                                     thus[c][:, :NH], AF.Sin,
                                     bias=bcos_sb[:, c:c + 1])
                nc.scalar.activation(uf[1][:, 0, c * NH:(c + 1) * NH],
                                     thus[c][:, :NH], AF.Sin,
                                     bias=bsin_sb[:, c:c + 1])

            nc.vector.tensor_scalar_mul(twov[:, 0, :], vf[1][:, 1, :], 2.0)
            nc.vector.tensor_scalar_mul(twov[:, 1, :], vf[1][:, 1, :], 2.0)
            nc.vector.tensor_scalar_mul(twou[:, 0, :], uf[1][:, 1, :], 2.0)
            nc.vector.tensor_scalar_mul(twou[:, 1, :], uf[1][:, 1, :], 2.0)

            sc = [scps.tile([128, NH], f32, tag=f"sc{mb}", name=f"sc{mb}")
                  for mb in range(4)]

            for j in range(1, FJ + 1):
                if j == 2:
                    # f_2 = 2c*f_1 - f_0 with f_0 = (0, 1):
                    # sin_2 = twoc*sin_1;  cos_2 = twoc*cos_1 - 1
                    tmpu = tmpp.tile([128, 2, NH * 2], f16, tag="tmpu")
                    tmpv = tmpp.tile([128, 2, N * 2], f16, tag="tmpv")
                    nc.vector.tensor_mul(tmpu[:], uf[1][:], twou[:])
                    nc.vector.tensor_mul(tmpv[:], vf[1][:], twov[:])
                    nc.vector.tensor_copy(uf[2][:, 0, :], tmpu[:, 0, :])
                    nc.vector.tensor_scalar_add(uf[2][:, 1, :], tmpu[:, 1, :],
                                                -1.0)
                    nc.vector.tensor_copy(vf[2][:, 0, :], tmpv[:, 0, :])
                    nc.vector.tensor_scalar_add(vf[2][:, 1, :], tmpv[:, 1, :],
                                                -1.0)
                elif j >= 3:
                    tmpu = tmpp.tile([128, 2, NH * 2], f16, tag="tmpu")
                    tmpv = tmpp.tile([128, 2, N * 2], f16, tag="tmpv")
                    nc.vector.tensor_mul(tmpu[:], uf[j - 1][:], twou[:])
                    nc.vector.tensor_sub(uf[j][:], tmpu[:], uf[j - 2][:])
                    nc.vector.tensor_mul(tmpv[:], vf[j - 1][:], twov[:])
                    nc.vector.tensor_sub(vf[j][:], tmpv[:], vf[j - 2][:])
                # scale u-features by Wa[a]*BJ[j-1] (both fn halves per op)
                for c in range(2):
                    nc.vector.tensor_scalar(
                        us[j][:, :, c * NH:(c + 1) * NH],
                        uf[j][:, :, c * NH:(c + 1) * NH],
                        wav_sb[:, c:c + 1], float(BJ[j - 1]),
                        MULT, MULT,
                    )
                # scoring: sin_u pairs cos_v, cos_u pairs sin_v
                for fn in range(2):
                    for c in range(2):
                        for mb in range(4):
                            nc.tensor.matmul(
                                sc[mb][:],
                                vf[j][:, 1 - fn,
                                      c * N + mb * 128: c * N + (mb + 1) * 128],
                                us[j][:, fn, c * NH:(c + 1) * NH],
                                start=(j == 1 and fn == 0 and c == 0),
                                stop=(j == FJ and fn == 1 and c == 1),
                                skip_group_check=True,
                            )

            attT = attp.tile([128, 4, NH], f16, tag="attT")
            out_sb = opool.tile([128, 2, D], f32, tag="out")
            fos = [fps.tile([128, D], f32, tag=f"fo{nb}", name=f"fo{nb}")
                   for nb in range(2)]
            for mb in range(4):
                nc.scalar.activation(
                    attT[:, mb, :], sc[mb][:], AF.Sigmoid, bias=sgb_sb[:, 0:1]
                )
                for nb in range(2):
                    nc.tensor.matmul(
                        fos[nb][:],
                        attT[:, mb, nb * 128:(nb + 1) * 128],
                        xkT_sb[:, mb, :],
                        start=(mb == 0),
                        stop=(mb == 3),
                        skip_group_check=True,
                    )
            for nb in range(2):
                nc.vector.tensor_copy(out_sb[:, nb, :], fos[nb][:])

            nc.sync.dma_start(
                out.ap().rearrange("(nb p) d -> p nb d", p=128), out_sb[:]
            )

    nc.compile()
    return nc


def _prep_inputs_v2(x, Wg1, Wg2, bg, Wa_w, Wa_b, ba):
    x = np.asarray(x, np.float32)
    w1s = (FS * np.asarray(Wg1, np.float32).T).astype(np.float16)
    w2s = (FS * np.asarray(Wg2, np.float32).T).astype(np.float16)
    bgv = FS * np.asarray(bg, np.float32)
    biasv = np.empty((128, 7), np.float32)
    biasv[:, 0:2] = bgv.reshape(2, 128).T
    biasv[:, 2:4] = bgv.reshape(2, 128).T + np.float32(np.pi / 2)
    biasv[:, 4:6] = np.asarray(Wa_w, np.float32).reshape(2, 128).T
    biasv[:, 6] = float(np.asarray(Wa_b).ravel()[0]) \
        + float(np.asarray(ba).ravel()[0])
    in_maps = []
    for c in range(NCORES):
        b, half = c // 2, c % 2
        xb = x[b]
        import os
        dt = np.float32 if int(os.environ.get("K_F32IN", "0")) else np.float16
        xb16 = xb.astype(dt)
        vin = np.ascontiguousarray(np.concatenate([w2s.astype(dt), xb16], axis=1))
        uin = np.ascontiguousarray(np.concatenate(
            [w1s.astype(dt), xb16[:, half * NH:(half + 1) * NH]], axis=1))
        in_maps.append({
            "vin": vin,
            "uin": uin,
            "biasv": np.ascontiguousarray(biasv),
            "xkT": np.ascontiguousarray(xb.T.astype(np.float16)),
        })
    return in_maps


def _prep_inputs(x, Wg1, Wg2, bg, Wa_w, Wa_b, ba):
    """Build the 8 per-core input maps (host-side slicing/packing only)."""
    x = np.asarray(x, np.float32)
    wg1T = np.ascontiguousarray(np.asarray(Wg1, np.float32).T)   # (96, 256)
    wg2T = np.ascontiguousarray(np.asarray(Wg2, np.float32).T)
    waT = np.ascontiguousarray(
        np.asarray(Wa_w, np.float32).reshape(2, 128).T.astype(np.float16)
    )                                                             # (128, 2)
    bgc = np.ascontiguousarray(np.asarray(bg, np.float32).reshape(2, 128).T)
    sgb = np.full((128, 1), float(np.asarray(Wa_b).ravel()[0])
                  + float(np.asarray(ba).ravel()[0]), np.float32)
    in_maps = []
    for c in range(NCORES):
        b, half = c // 2, c % 2
        xb = x[b]                                                 # (96, 512)
        in_maps.append({
            "xq": np.ascontiguousarray(xb[:, half * NH : (half + 1) * NH]),
            "xk": np.ascontiguousarray(xb),
            "xkT": np.ascontiguousarray(xb.T.astype(np.float16)),
            "wg1T": wg1T,
            "wg2T": wg2T,
            "waT": waT,
            "bgc": bgc,
            "sgb": sgb,
        })
    return in_maps


def _run(inputs, trace=False):
    from concourse.bass_utils import run_bass_kernel_spmd

    if "nc" not in _cache:
        _cache["nc"] = _build_nc_v2() if VERSION == 2 else _build_nc()
    nc = _cache["nc"]
    in_maps = (_prep_inputs_v2 if VERSION == 2 else _prep_inputs)(**inputs)
    res = run_bass_kernel_spmd(
        nc, in_maps, core_ids=list(range(NCORES)), trace=trace
    )
    out = np.empty((B, N, D), np.float32)
    for c in range(NCORES):
        b, half = c // 2, c % 2
        out[b, half * NH : (half + 1) * NH] = res.results[c]["out"]
    return out, res


def kernel(**inputs):
    out, _ = _run(inputs, trace=False)
    return out
